# revision 30
# baseline (speedup 1.0000x reference)
"""GCN link predictor on 8 TRN2 NeuronCores.

Strategy (1D node partition, dst-sharded SPMM, pull-mode gathers):
  - x is replicated (transposed, bf16); every core computes the full
    h1 = x @ W1 gather table [100096, 128] bf16 (256B rows).
  - adjacency edges are sharded by dst owner (12500 nodes/core); within a
    core, edges are grouped by (128-row dst block, 32768-row src chunk) so
    each dma_gather call uses int16 indices against a single chunk base.
  - segment-sum = PE matmul with one-hots built in BATCHED DVE
    tensor_tensor ops (broadcast APs) per superblock — avoids the ~2us
    fixed cost of per-tile tensor_scalar.
  - layer 2: h2 = z1 @ W2 locally, AllGather h2 (bf16), expand into a
    256B-row table, same SPMM -> z2 (bf16).
  - decoder: edges sharded by dst owner, sorted by (dst window, src
    chunk).  dst side = PE select from SBUF-resident local z2 windows
    (one-hot streamed from host, zero descriptors); src side = transposed
    dma_gather (A^T columns); DVE product + PE ones-reduction into PSUM
    score rows; periodic flush to SBUF.
"""
import sys
import os

sys.path.insert(0, "/opt/trn_rl_repo")

import numpy as np
import ml_dtypes
from contextlib import ExitStack

from concourse import bass, bacc, tile, bass_utils
import concourse.mybir as mybir


def _install_ntff_hook():
    """Provide antenv.axon_hooks (missing in this image) so that
    run_bass_kernel_spmd(trace=True) can capture NTFF profiles via the
    axon PJRT .so — mirrors trn_agent_boot's ctypes shim."""
    if "antenv.axon_hooks" in sys.modules:
        return
    import types, ctypes, contextlib
    import antenv

    mod = types.ModuleType("antenv.axon_hooks")
    holder = {}
    mod.set_axon_ntff_profile_hook = lambda h: holder.__setitem__("h", h)
    mod.get_axon_ntff_profile_hook = lambda: holder.get("h")
    sys.modules["antenv.axon_hooks"] = mod
    antenv.axon_hooks = mod

    so_path = "/opt/axon/libaxon_pjrt.so"
    if not os.path.exists(so_path):
        return
    lib = ctypes.CDLL(so_path)
    if not hasattr(lib, "axon_start_nrt_profile"):
        return
    lib.axon_start_nrt_profile.argtypes = [ctypes.POINTER(ctypes.c_int64),
                                           ctypes.c_size_t]
    lib.axon_start_nrt_profile.restype = ctypes.c_int64
    lib.axon_stop_nrt_profile.argtypes = [ctypes.c_char_p]
    lib.axon_stop_nrt_profile.restype = ctypes.c_int64

    @contextlib.contextmanager
    def _hook(output_dir, device_ids):
        import jax
        jax.devices()
        if device_ids:
            ids = (ctypes.c_int64 * len(device_ids))(*device_ids)
            rc = lib.axon_start_nrt_profile(ids, len(device_ids))
        else:
            rc = lib.axon_start_nrt_profile(None, 0)
        if rc != 0:
            raise RuntimeError(f"axon_start_nrt_profile rc={rc}")
        try:
            yield
        finally:
            n = lib.axon_stop_nrt_profile(str(output_dir).encode())
            print(f"profile: {n} file(s) written to {output_dir}",
                  file=sys.stderr)

    mod.set_axon_ntff_profile_hook(_hook)


_install_ntff_hook()

F32 = mybir.dt.float32
BF16 = mybir.dt.bfloat16
I16 = mybir.dt.int16
BF = ml_dtypes.bfloat16

N_NODES = 100000
D_IN = 256
D_HID = 64
D_EMB = 32
ADJ_NNZ = 3200000
N_EDGES = 2000000
NCORE = 8
P_NODES = N_NODES // NCORE          # 12500
NODES_PAD = 782 * 128               # 100096
ZROWS = 98 * 128                    # 12544 (per-core z rows, padded)
NBLK = 98                           # dst blocks per core (last has 84 rows)
CHUNK = 32768
NCHUNK = 4
SUPER = 4                           # dst blocks per superblock
NSUPER = (NBLK + SUPER - 1) // SUPER
DEC_SUPERW = 3                      # decoder dst windows per gather group
DEC_NSW = (NBLK + DEC_SUPERW - 1) // DEC_SUPERW
NPAIR4 = 25000                      # pack-4 rows of a [N,32]bf16 table

LAST_RESULT = None                  # BassKernelResults of the last run


def _wrap16(idx):
    """idx j -> partition j%16, col j//16, replicated to 128 partitions."""
    n = len(idx)
    assert n % 16 == 0
    a = idx.reshape(n // 16, 16).T
    return np.tile(a, (8, 1)).astype(np.int16)


def _prep_spmm(adj_src, adj_dst, adj_val):
    """Shard + sort adjacency; build per-core gather/meta streams.

    Returns (T_bc [98,4] common tile counts, NT, per-core dict with idx_w,
    dst_meta (bf16), val_meta (bf16)).
    """
    owner = adj_dst // P_NODES
    cores = []
    for m in range(NCORE):
        sel = owner == m
        src = adj_src[sel].astype(np.int64)
        ldst = (adj_dst[sel] - m * P_NODES).astype(np.int64)
        val = adj_val[sel]
        blk = ldst >> 7
        chk = src // CHUNK
        order = np.lexsort((chk, blk))
        src, ldst, val, blk, chk = (a[order] for a in (src, ldst, val, blk, chk))
        key = blk * NCHUNK + chk
        cnt = np.bincount(key, minlength=NBLK * NCHUNK).reshape(NBLK, NCHUNK)
        starts = np.zeros(NBLK * NCHUNK + 1, np.int64)
        np.cumsum(cnt.ravel(), out=starts[1:])
        cores.append(dict(src=src, ldst=ldst, val=val, cnt=cnt, starts=starts))

    cnt_max = np.maximum.reduce([c["cnt"] for c in cores])
    T_bc = -(-cnt_max // 128)  # ceil
    NT = int(T_bc.sum())

    for c in cores:
        idx_stream = np.zeros(NT * 128, np.int16)
        dst_stream = np.zeros(NT * 128, np.float32)
        val_stream = np.zeros(NT * 128, np.float32)
        pos = 0
        for sb in range(NSUPER):
            blocks = range(sb * SUPER, min((sb + 1) * SUPER, NBLK))
            for ch in range(NCHUNK):
                for b in blocks:
                    t = int(T_bc[b, ch])
                    if t == 0:
                        continue
                    s = c["starts"][b * NCHUNK + ch]
                    e = c["starts"][b * NCHUNK + ch + 1]
                    n = e - s
                    idx_stream[pos:pos + n] = (c["src"][s:e] - ch * CHUNK).astype(np.int16)
                    dst_stream[pos:pos + n] = (c["ldst"][s:e] & 127).astype(np.float32)
                    val_stream[pos:pos + n] = c["val"][s:e]
                    pos += t * 128
        assert pos == NT * 128
        # wrap idx per call (call = (sb, ch) contiguous span)
        cols = []
        p = 0
        for sb in range(NSUPER):
            blocks = range(sb * SUPER, min((sb + 1) * SUPER, NBLK))
            for ch in range(NCHUNK):
                t = int(T_bc[list(blocks), ch].sum())
                if t == 0:
                    continue
                cols.append(_wrap16(idx_stream[p:p + t * 128]))
                p += t * 128
        c["idx_w"] = np.concatenate(cols, axis=1)
        c["dst_meta"] = dst_stream.reshape(NT, 128).T.astype(BF).copy()
        c["val_meta"] = val_stream.reshape(NT, 128).T.astype(BF).copy()
        for k in ("src", "ldst", "val", "cnt", "starts"):
            del c[k]
    return T_bc, NT, cores


def _prep_decoder(edge_index):
    """Shard decoder edges by dst owner; group by (dst window, src%4).

    z2 table is the pack-4 view [25000, 128] of [100000, 32] bf16; the
    gather idx is src//4 (single int16 chunk) and src%4 selects the
    32-col slice of the gathered element (static per cell).
    """
    src_g = edge_index[0].astype(np.int64)
    dst_g = edge_index[1].astype(np.int64)
    owner = dst_g // P_NODES
    cores = []
    for m in range(NCORE):
        sel = owner == m
        src = src_g[sel]
        ldst = dst_g[sel] - m * P_NODES
        eid = np.nonzero(sel)[0]
        w = ldst >> 7                      # 98 windows
        c = src % 4                        # parity cell
        sw = w // DEC_SUPERW
        order = np.lexsort((w, c, sw))
        src, ldst, eid, w, c, sw = (a[order] for a in (src, ldst, eid, w, c, sw))
        key = (w * 4 + c)
        cnt = np.bincount(key, minlength=NBLK * 4).reshape(NBLK, 4)
        cores.append(dict(src=src, ldst=ldst, eid=eid, cnt=cnt))

    cnt_max = np.maximum.reduce([c["cnt"] for c in cores])
    T_wc = -(-cnt_max // 128)              # [98, 4] tiles
    DEC_NT = int(T_wc.sum())

    for cd in cores:
        idx_stream = np.zeros(DEC_NT * 128, np.int16)
        oh = np.zeros((128, DEC_NT * 128), BF)
        emap = np.full(DEC_NT * 128, -1, np.int64)
        # per-(w,par) start offsets in this core's sorted arrays
        starts = {}
        pos0 = 0
        for isw in range(DEC_NSW):
            ws = range(isw * DEC_SUPERW, min((isw + 1) * DEC_SUPERW, NBLK))
            for par in range(4):
                for iw in ws:
                    starts[(iw, par)] = pos0
                    pos0 += int(cd["cnt"][iw, par])
        pos = 0
        for isw in range(DEC_NSW):
            ws = range(isw * DEC_SUPERW, min((isw + 1) * DEC_SUPERW, NBLK))
            for par in range(4):
                for iw in ws:
                    t = int(T_wc[iw, par])
                    if t == 0:
                        continue
                    s = starts[(iw, par)]
                    n = int(cd["cnt"][iw, par])
                    idx_stream[pos:pos + n] = (cd["src"][s:s + n] // 4).astype(np.int16)
                    loff = (cd["ldst"][s:s + n] & 127).astype(np.int64)
                    oh[loff, np.arange(pos, pos + n)] = BF(1.0)
                    emap[pos:pos + n] = cd["eid"][s:s + n]
                    pos += t * 128
        assert pos == DEC_NT * 128
        # wrap idx per (sw) call
        cols = []
        p = 0
        for isw in range(DEC_NSW):
            ws = list(range(isw * DEC_SUPERW, min((isw + 1) * DEC_SUPERW, NBLK)))
            t = int(T_wc[ws, :].sum())
            if t == 0:
                continue
            cols.append(_wrap16(idx_stream[p:p + t * 128]))
            p += t * 128
        cd["idx_w"] = np.concatenate(cols, axis=1)
        cd["ohdst"] = oh
        cd["emap"] = emap
        for k in ("src", "ldst", "eid", "cnt"):
            del cd[k]
    return T_wc, DEC_NT, cores


def _prep_spmm2(adj_src, adj_dst, adj_val):
    """Layer-2 prep for the pack-4 table: edges grouped by dst block only
    (single idx chunk, src//4 < 25000); parity stream src%4 for the
    masked-stationary matmul."""
    owner = adj_dst // P_NODES
    cores = []
    for m in range(NCORE):
        sel = owner == m
        src = adj_src[sel].astype(np.int64)
        ldst = (adj_dst[sel] - m * P_NODES).astype(np.int64)
        val = adj_val[sel]
        blk = ldst >> 7
        order = np.argsort(blk, kind="stable")
        src, ldst, val, blk = src[order], ldst[order], val[order], blk[order]
        cnt = np.bincount(blk, minlength=NBLK)
        starts = np.zeros(NBLK + 1, np.int64)
        np.cumsum(cnt, out=starts[1:])
        cores.append(dict(src=src, ldst=ldst, val=val, cnt=cnt, starts=starts))

    cnt_max = np.maximum.reduce([c["cnt"] for c in cores])
    T_b = -(-cnt_max // 128)               # [98]
    NT2 = int(T_b.sum())

    for c in cores:
        idx_stream = np.zeros(NT2 * 128, np.int16)
        dst_stream = np.zeros(NT2 * 128, np.float32)
        val_stream = np.zeros(NT2 * 128, np.float32)
        par_stream = np.zeros(NT2 * 128, np.float32)
        pos = 0
        for b in range(NBLK):
            s, e = c["starts"][b], c["starts"][b + 1]
            n = e - s
            idx_stream[pos:pos + n] = (c["src"][s:e] // 4).astype(np.int16)
            dst_stream[pos:pos + n] = (c["ldst"][s:e] & 127).astype(np.float32)
            val_stream[pos:pos + n] = c["val"][s:e]
            par_stream[pos:pos + n] = (c["src"][s:e] % 4).astype(np.float32)
            pos += int(T_b[b]) * 128
        assert pos == NT2 * 128
        cols = []
        p = 0
        for b in range(NBLK):
            t = int(T_b[b])
            if t == 0:
                continue
            cols.append(_wrap16(idx_stream[p:p + t * 128]))
            p += t * 128
        c["idx2_w"] = np.concatenate(cols, axis=1)
        c["dst2"] = dst_stream.reshape(NT2, 128).T.astype(BF).copy()
        c["val2"] = val_stream.reshape(NT2, 128).T.astype(BF).copy()
        c["par2"] = par_stream.reshape(NT2, 128).T.astype(BF).copy()
        for k in ("src", "ldst", "val", "cnt", "starts"):
            del c[k]
    return T_b, NT2, cores


def _build(T_bc, NT, T_b2, NT2, T_wc, DEC_NT, idx_cols, idx2_cols,
           dec_idx_cols):
    nc = bacc.Bacc("TRN2", target_bir_lowering=False, debug=False,
                   num_devices=NCORE)

    xT_d = nc.dram_tensor("xT", [D_IN, ZROWS], BF16, kind="ExternalInput")
    w1_d = nc.dram_tensor("W1", [D_IN, D_HID], BF16, kind="ExternalInput")
    w2_d = nc.dram_tensor("W2", [D_HID, D_EMB], BF16, kind="ExternalInput")
    b1t_d = nc.dram_tensor("b1t", [128, D_HID], BF16, kind="ExternalInput")
    b2blk_d = nc.dram_tensor("b2blk", [128, 128], BF16, kind="ExternalInput")
    row0s_d = nc.dram_tensor("row0s", [128, 128], BF16, kind="ExternalInput")
    c32_d = nc.dram_tensor("c32", [128, 128], BF16, kind="ExternalInput")
    iota_d = nc.dram_tensor("iota", [128, 128], BF16, kind="ExternalInput")
    ident_d = nc.dram_tensor("ident", [128, 128], BF16, kind="ExternalInput")
    idx_d = nc.dram_tensor("idx", [128, idx_cols], I16, kind="ExternalInput")
    dstm_d = nc.dram_tensor("dstm", [128, NT], BF16, kind="ExternalInput")
    valm_d = nc.dram_tensor("valm", [128, NT], BF16, kind="ExternalInput")
    idx2_d = nc.dram_tensor("idx2", [128, idx2_cols], I16, kind="ExternalInput")
    dst2_d = nc.dram_tensor("dst2", [128, NT2], BF16, kind="ExternalInput")
    val2_d = nc.dram_tensor("val2", [128, NT2], BF16, kind="ExternalInput")
    par2_d = nc.dram_tensor("par2", [128, NT2], BF16, kind="ExternalInput")
    didx_d = nc.dram_tensor("didx", [128, dec_idx_cols], I16, kind="ExternalInput")
    doh_d = nc.dram_tensor("doh", [128, DEC_NT * 128], BF16, kind="ExternalInput")
    scores_d = nc.dram_tensor("scores", [128, DEC_NT], F32,
                              kind="ExternalOutput")

    # internal DRAM
    h1loc_d = nc.dram_tensor("h1loc", [ZROWS, D_HID], BF16, kind="Internal")
    h1full_d = nc.dram_tensor("h1full", [784 * 128, D_HID], BF16,
                              kind="Internal", addr_space="Shared")
    h1pad_d = nc.dram_tensor("h1pad", [NODES_PAD, 128], BF16, kind="Internal")
    z1_d = nc.dram_tensor("z1", [ZROWS, 128], BF16, kind="Internal")
    h2loc_d = nc.dram_tensor("h2loc", [ZROWS, D_EMB], BF16, kind="Internal")
    h2full_d = nc.dram_tensor("h2full", [N_NODES, D_EMB], BF16,
                              kind="Internal", addr_space="Shared")
    z2loc_d = nc.dram_tensor("z2loc", [ZROWS, D_EMB], BF16, kind="Internal")
    z2full_d = nc.dram_tensor("z2full", [N_NODES, D_EMB], BF16,
                              kind="Internal", addr_space="Shared")

    rg = [list(range(NCORE))]

    def _expand(tc, pool_, src_d, dst_d, width, dt):
        """Copy [N_NODES, width] rows into the 256B-stride table cols 0:width
        via SBUF bounce tiles."""
        R = 8192
        for r0 in range(0, N_NODES, R):
            n = min(R, N_NODES - r0)
            nb = -(-n // 128)
            t = pool_.tile([128, nb, width], dt, tag="expand")
            if n % 128:
                full = n // 128
                if full:
                    nc.sync.dma_start(
                        t[:, 0:full, :],
                        src_d[r0:r0 + full * 128, :]
                        .rearrange("(j p) f -> p j f", p=128))
                rem = n - full * 128
                nc.sync.dma_start(t[0:rem, full, :], src_d[r0 + full * 128:r0 + n, :])
                if full:
                    nc.sync.dma_start(
                        dst_d[r0:r0 + full * 128, 0:width]
                        .rearrange("(j p) f -> p j f", p=128),
                        t[:, 0:full, :])
                nc.sync.dma_start(dst_d[r0 + full * 128:r0 + n, 0:width],
                                  t[0:rem, full, :])
            else:
                nc.sync.dma_start(
                    t[:], src_d[r0:r0 + n, :]
                    .rearrange("(j p) f -> p j f", p=128))
                nc.sync.dma_start(
                    dst_d[r0:r0 + n, 0:width]
                    .rearrange("(j p) f -> p j f", p=128),
                    t[:])

    # per-call tile counts for spmm gathers
    def spmm_calls():
        out = []
        for sb in range(NSUPER):
            blocks = list(range(sb * SUPER, min((sb + 1) * SUPER, NBLK)))
            for ch in range(NCHUNK):
                t = int(T_bc[blocks, ch].sum())
                if t:
                    out.append((sb, ch, blocks, t))
        return out

    CALLS = spmm_calls()
    call_tile_base = {}
    tb = 0
    for (sb, ch, blocks, t) in CALLS:
        call_tile_base[(sb, ch)] = tb
        tb += t
    assert tb == NT

    def spmm_phase(tc, pool, iota_sb, ident_sb, table_ap, out_w, bias_tile,
                   relu, out_dtype, z_out_d, tag):
        nc_ = tc.nc
        with ExitStack() as ctx:
            lp = ctx.enter_context(tc.tile_pool(name=f"sp_{tag}", bufs=3))
            ohp = ctx.enter_context(tc.tile_pool(name=f"oh_{tag}", bufs=2))
            gp = ctx.enter_context(tc.tile_pool(name=f"g_{tag}", bufs=8))
            pp = ctx.enter_context(
                tc.tile_pool(name=f"ps_{tag}", bufs=6, space="PSUM"))
            bt_sb = pool.tile([128, out_w], BF16, tag=f"bt_{tag}")
            nc_.sync.dma_start(bt_sb[:], bias_tile[:])

            icol = 0
            for sb in range(NSUPER):
                blocks = list(range(sb * SUPER, min((sb + 1) * SUPER, NBLK)))
                sb_tiles = int(T_bc[blocks, :].sum())
                sb_tile0 = call_tile_base[(sb, [ch for ch in range(NCHUNK)
                                                if (sb, ch) in call_tile_base][0])]
                dst_sb = lp.tile([128, sb_tiles], BF16, tag="dstm")
                val_sb = lp.tile([128, sb_tiles], BF16, tag="valm")
                nc_.sync.dma_start(dst_sb[:], dstm_d[:, sb_tile0:sb_tile0 + sb_tiles])
                nc_.sync.dma_start(val_sb[:], valm_d[:, sb_tile0:sb_tile0 + sb_tiles])
                idx_sb = lp.tile([128, sb_tiles * 8], I16, tag="idx")
                nc_.sync.dma_start(idx_sb[:], idx_d[:, icol:icol + sb_tiles * 8])

                gath = {}
                ic_local = 0
                for ch in range(NCHUNK):
                    if (sb, ch) not in call_tile_base:
                        continue
                    t = int(T_bc[blocks, ch].sum())
                    rows = min(CHUNK, NODES_PAD - ch * CHUNK)
                    g = gp.tile([128, t, 128], BF16, tag="gath")
                    nc_.gpsimd.dma_gather(
                        out_ap=g[:],
                        in_ap=table_ap[ch * CHUNK:ch * CHUNK + rows, :],
                        idxs_ap=idx_sb[:, ic_local:ic_local + t * 8],
                        num_idxs=t * 128,
                        num_idxs_reg=t * 128,
                        elem_size=128,
                        single_packet=False,
                    )
                    gath[ch] = g
                    ic_local += t * 8
                icol += sb_tiles * 8

                # batched one-hot build for the whole superblock
                oh_sb = ohp.tile([128, sb_tiles, 128], BF16, tag="oh")
                i_b = iota_sb[:].unsqueeze(1).broadcast_to([128, sb_tiles, 128])
                d_b = dst_sb[:].unsqueeze(2).broadcast_to([128, sb_tiles, 128])
                v_b = val_sb[:].unsqueeze(2).broadcast_to([128, sb_tiles, 128])
                nc_.vector.tensor_tensor(oh_sb[:], i_b, d_b,
                                         mybir.AluOpType.is_equal)
                nc_.vector.tensor_tensor(oh_sb[:], oh_sb[:], v_b,
                                         mybir.AluOpType.mult)

                zwide = lp.tile([128, len(blocks), out_w], out_dtype, tag="zw")
                for bi, b in enumerate(blocks):
                    ps = pp.tile([128, out_w], F32, tag="ps")
                    first = True
                    for ch in range(NCHUNK):
                        if (sb, ch) not in call_tile_base:
                            continue
                        off = int(T_bc[blocks[:bi], ch].sum()) if bi else 0
                        gtile0 = call_tile_base[(sb, ch)] + off
                        for ti in range(int(T_bc[b, ch])):
                            mcol = gtile0 + ti - sb_tile0
                            nc_.tensor.matmul(
                                ps[:], oh_sb[:, mcol, :],
                                gath[ch][:, off + ti, 0:out_w],
                                start=first, stop=False)
                            first = False
                    nc_.tensor.matmul(ps[:], ident_sb[:], bt_sb[:],
                                      start=first, stop=True)
                    func = (mybir.ActivationFunctionType.Relu if relu
                            else mybir.ActivationFunctionType.Copy)
                    nc_.scalar.activation(zwide[:, bi, :], ps[:], func)
                nc_.sync.dma_start(
                    z_out_d[sb * SUPER * 128:
                            (sb * SUPER + len(blocks)) * 128, :]
                    .rearrange("(j p) f -> p j f", p=128),
                    zwide[:],
                )

    with tile.TileContext(nc) as tc:
        with ExitStack() as octx:
            pool = octx.enter_context(tc.tile_pool(name="const", bufs=1))
            iota_sb = pool.tile([128, 128], BF16, tag="iota")
            ident_sb = pool.tile([128, 128], BF16, tag="ident")
            nc.sync.dma_start(iota_sb[:], iota_d[:])
            nc.sync.dma_start(ident_sb[:], ident_d[:])

            # ---- Phase A: h1 shard = x[:, my 98 blocks] @ W1; AllGather ----
            with ExitStack() as ctx:
                ap = ctx.enter_context(tc.tile_pool(name="pA", bufs=3))
                app = ctx.enter_context(
                    tc.tile_pool(name="pAp", bufs=8, space="PSUM"))
                w1_sb = pool.tile([128, 2, D_HID], BF16, tag="w1")
                nc.sync.dma_start(
                    w1_sb[:], w1_d[:].rearrange("(k p) f -> p k f", p=128))
                PB = 8  # node blocks per panel
                for p0 in range(0, NBLK, PB):
                    nb = min(PB, NBLK - p0)
                    n0 = p0 * 128
                    xt0 = ap.tile([128, nb * 128], BF16, tag="xt0")
                    xt1 = ap.tile([128, nb * 128], BF16, tag="xt1")
                    nc.sync.dma_start(xt0[:], xT_d[0:128, n0:n0 + nb * 128])
                    nc.sync.dma_start(xt1[:], xT_d[128:256, n0:n0 + nb * 128])
                    hw = ap.tile([128, nb, D_HID], BF16, tag="hw")
                    for j in range(nb):
                        ps = app.tile([128, D_HID], F32, tag="psA")
                        nc.tensor.matmul(ps[:], xt0[:, j * 128:(j + 1) * 128],
                                         w1_sb[:, 0, :], start=True, stop=False)
                        nc.tensor.matmul(ps[:], xt1[:, j * 128:(j + 1) * 128],
                                         w1_sb[:, 1, :], start=False, stop=True)
                        nc.scalar.activation(hw[:, j, :], ps[:],
                                             mybir.ActivationFunctionType.Copy)
                    nc.sync.dma_start(
                        h1loc_d[p0 * 128:(p0 + nb) * 128, :]
                        .rearrange("(j p) f -> p j f", p=128),
                        hw[:],
                    )
                nc.gpsimd.collective_compute(
                    "AllGather", mybir.AluOpType.bypass, replica_groups=rg,
                    ins=[h1loc_d[:]], outs=[h1full_d.ap()])
                _expand(tc, ap, h1full_d, h1pad_d, D_HID, BF16)

            # ---- Phase B: SPMM1 -> z1 (relu) ----
            spmm_phase(tc, pool, iota_sb, ident_sb, h1pad_d[:], D_HID, b1t_d,
                       True, BF16, z1_d[:, 0:D_HID], "b")

            # ---- Phase C: h2 = z1 @ W2; AllGather; expand to table2 ----
            with ExitStack() as ctx:
                cp = ctx.enter_context(tc.tile_pool(name="pC", bufs=3))
                cpp = ctx.enter_context(
                    tc.tile_pool(name="pCp", bufs=4, space="PSUM"))
                z1T = cp.tile([128, ZROWS], BF16, tag="z1T")
                nc.sync.dma_start(z1T[:], z1_d[:], transpose=True)
                w2_sb = pool.tile([D_HID, D_EMB], BF16, tag="w2")
                nc.sync.dma_start(w2_sb[:], w2_d[:])
                PB = 8
                for p0 in range(0, NBLK, PB):
                    nb = min(PB, NBLK - p0)
                    hw = cp.tile([128, nb, D_EMB], BF16, tag="h2w")
                    for j in range(nb):
                        b = p0 + j
                        ps = cpp.tile([128, D_EMB], F32, tag="psC")
                        nc.tensor.matmul(ps[:], z1T[0:D_HID, b * 128:(b + 1) * 128],
                                         w2_sb[:], start=True, stop=True)
                        nc.scalar.activation(hw[:, j, :], ps[:],
                                             mybir.ActivationFunctionType.Copy)
                    nc.sync.dma_start(
                        h2loc_d[p0 * 128:(p0 + nb) * 128, :]
                        .rearrange("(j p) f -> p j f", p=128),
                        hw[:],
                    )
                nc.gpsimd.collective_compute(
                    "AllGather", mybir.AluOpType.bypass, replica_groups=rg,
                    ins=[h2loc_d[0:P_NODES, :]], outs=[h2full_d.ap()])

            # ---- Phase D: SPMM2 -> z2 via pack-4 table (h2full view) ----
            tab2_ap = h2full_d[:].rearrange("(j k) f -> j (k f)", k=4)
            with ExitStack() as ctx:
                lp = ctx.enter_context(tc.tile_pool(name="sp_d", bufs=3))
                ohp = ctx.enter_context(tc.tile_pool(name="oh_d", bufs=2))
                gp = ctx.enter_context(tc.tile_pool(name="g_d", bufs=6))
                pp = ctx.enter_context(
                    tc.tile_pool(name="ps_d", bufs=6, space="PSUM"))
                c32_sb = pool.tile([128, 128], BF16, tag="c32")
                b2blk_sb = pool.tile([128, 128], BF16, tag="b2blk")
                row0s_sb = pool.tile([128, 128], BF16, tag="row0s")
                nc.sync.dma_start(c32_sb[:], c32_d[:])
                nc.sync.dma_start(b2blk_sb[:], b2blk_d[:])
                nc.sync.dma_start(row0s_sb[:], row0s_d[:])
                icol2 = 0
                tpos2 = 0
                PB = 4
                zwide = None
                for b in range(NBLK):
                    t = int(T_b2[b])
                    dst_sb = lp.tile([128, t], BF16, tag="dst2")
                    val_sb = lp.tile([128, t], BF16, tag="val2")
                    par_sb = lp.tile([128, t], BF16, tag="par2")
                    nc.sync.dma_start(dst_sb[:], dst2_d[:, tpos2:tpos2 + t])
                    nc.sync.dma_start(val_sb[:], val2_d[:, tpos2:tpos2 + t])
                    nc.sync.dma_start(par_sb[:], par2_d[:, tpos2:tpos2 + t])
                    idx_sb = lp.tile([128, t * 8], I16, tag="idx2")
                    nc.sync.dma_start(idx_sb[:], idx2_d[:, icol2:icol2 + t * 8])
                    icol2 += t * 8
                    g4 = gp.tile([128, t, 128], BF16, tag="g4")
                    nc.gpsimd.dma_gather(
                        out_ap=g4[:], in_ap=tab2_ap[0:NPAIR4, :],
                        idxs_ap=idx_sb[:], num_idxs=t * 128,
                        num_idxs_reg=t * 128, elem_size=128,
                        single_packet=False)
                    # batched DVE: binary dst one-hot + parity/val mask
                    ohb = ohp.tile([128, t, 128], BF16, tag="ohb")
                    g4m = ohp.tile([128, t, 128], BF16, tag="g4m")
                    i_b = iota_sb[:].unsqueeze(1).broadcast_to([128, t, 128])
                    c_b = c32_sb[:].unsqueeze(1).broadcast_to([128, t, 128])
                    d_b = dst_sb[:].unsqueeze(2).broadcast_to([128, t, 128])
                    v_b = val_sb[:].unsqueeze(2).broadcast_to([128, t, 128])
                    p_b = par_sb[:].unsqueeze(2).broadcast_to([128, t, 128])
                    nc.vector.tensor_tensor(ohb[:], i_b, d_b,
                                            mybir.AluOpType.is_equal)
                    nc.vector.tensor_tensor(g4m[:], c_b, p_b,
                                            mybir.AluOpType.is_equal)
                    nc.vector.tensor_tensor(g4m[:], g4m[:], v_b,
                                            mybir.AluOpType.mult)
                    nc.vector.tensor_tensor(g4m[:], g4m[:], g4[:],
                                            mybir.AluOpType.mult)
                    ps = pp.tile([128, 128], F32, tag="psD")
                    for ti in range(t):
                        nc.tensor.matmul(ps[:], g4m[:, ti, :], ohb[:, ti, :],
                                         start=(ti == 0), stop=False)
                    nc.tensor.matmul(ps[:], b2blk_sb[:], row0s_sb[:],
                                     start=False, stop=True)
                    # z2T_blk[f, d] = sum_q ps[q*32+f, d]; transpose to [d, f]
                    pcop = lp.tile([128, 128], BF16, tag="pcop")
                    nc.scalar.activation(pcop[:], ps[:],
                                         mybir.ActivationFunctionType.Copy)
                    zt = lp.tile([32, 128], BF16, tag="zt")
                    za = lp.tile([32, 128], BF16, tag="za")
                    zb = lp.tile([32, 128], BF16, tag="zb")
                    nc.vector.tensor_tensor(za[:], pcop[0:32, :], pcop[32:64, :],
                                            mybir.AluOpType.add)
                    nc.vector.tensor_tensor(zb[:], pcop[64:96, :], pcop[96:128, :],
                                            mybir.AluOpType.add)
                    nc.vector.tensor_tensor(zt[:], za[:], zb[:],
                                            mybir.AluOpType.add)
                    if b % PB == 0:
                        zwide = lp.tile([128, PB, D_EMB], BF16, tag="zw2")
                    for q in range(4):
                        nc.vector.transpose(
                            zwide[q * 32:(q + 1) * 32, b % PB, :],
                            zt[:, q * 32:(q + 1) * 32])
                    if b % PB == PB - 1 or b == NBLK - 1:
                        b0 = (b // PB) * PB
                        nbw = b - b0 + 1
                        nc.sync.dma_start(
                            z2loc_d[b0 * 128:(b0 + nbw) * 128, :]
                            .rearrange("(j p) f -> p j f", p=128),
                            zwide[:, 0:nbw, :],
                        )
                    tpos2 += t

            # AllGather z2 (bf16); decoder reads the pack-4 view directly
            nc.gpsimd.collective_compute(
                "AllGather", mybir.AluOpType.bypass, replica_groups=rg,
                ins=[z2loc_d[0:P_NODES, :]], outs=[z2full_d.ap()])
            tab3_ap = z2full_d[:].rearrange("(j k) f -> j (k f)", k=4)

            # ---- Phase E: decoder ----
            GRP = 8
            with ExitStack() as ctx:
                ep = ctx.enter_context(tc.tile_pool(name="pE", bufs=3))
                egp = ctx.enter_context(tc.tile_pool(name="pEg", bufs=4))
                epp = ctx.enter_context(
                    tc.tile_pool(name="pEp", bufs=4, space="PSUM"))
                # local z2 windows, partition = ldst & 127
                z2w_sb = pool.tile([128, NBLK, D_EMB], BF16, tag="z2w")
                nc.sync.dma_start(
                    z2w_sb[:],
                    z2loc_d[:].rearrange("(j p) f -> p j f", p=128))
                # scores accumulate in SBUF [128, DEC_NT] (edge = t*128 + p)
                sc_sb = pool.tile([128, DEC_NT], F32, tag="scsb")

                tile_pos = 0          # global tile counter (stream order)
                icol = 0
                for isw in range(DEC_NSW):
                    ws = list(range(isw * DEC_SUPERW,
                                    min((isw + 1) * DEC_SUPERW, NBLK)))
                    t = int(T_wc[ws, :].sum())
                    if t == 0:
                        continue
                    didx_sb = ep.tile([128, t * 8], I16, tag="didx")
                    nc.sync.dma_start(didx_sb[:],
                                      didx_d[:, icol:icol + t * 8])
                    icol += t * 8
                    at = egp.tile([128, t, 128], BF16, tag="at")
                    nc.gpsimd.dma_gather(
                        out_ap=at[:],
                        in_ap=tab3_ap[0:NPAIR4, :],
                        idxs_ap=didx_sb[:],
                        num_idxs=t * 128, num_idxs_reg=t * 128,
                        elem_size=128, single_packet=False)
                    oh_sb = ep.tile([128, t * 128], BF16, tag="doh")
                    nc.sync.dma_start(
                        oh_sb[:],
                        doh_d[:, tile_pos * 128:(tile_pos + t) * 128])
                    # (window, parity) of each tile in this call, stream order
                    tl = 0
                    for par in range(4):
                        for iw in ws:
                            tw = int(T_wc[iw, par])
                            if tw == 0:
                                continue
                            for g0 in range(0, tw, GRP):
                                gn = min(GRP, tw - g0)
                                bt2 = epp.tile([128, GRP, D_EMB], F32,
                                               tag="bt2")
                                for j in range(gn):
                                    tj = tl + g0 + j
                                    nc.tensor.matmul(
                                        bt2[:, j, :],
                                        oh_sb[:, tj * 128:(tj + 1) * 128],
                                        z2w_sb[:, iw, :],
                                        start=True, stop=True)
                                prod = ep.tile([128, GRP, D_EMB], BF16,
                                               tag="prod")
                                nc.vector.tensor_tensor(
                                    prod[:, 0:gn, :],
                                    at[:, tl + g0:tl + g0 + gn,
                                       par * D_EMB:(par + 1) * D_EMB],
                                    bt2[:, 0:gn, :], mybir.AluOpType.mult)
                                nc.vector.tensor_reduce(
                                    sc_sb[:, tile_pos + tl + g0:
                                          tile_pos + tl + g0 + gn],
                                    prod[:, 0:gn, :], mybir.AxisListType.X,
                                    mybir.AluOpType.add)
                            tl += tw
                    assert tl == t
                    tile_pos += t
                nc.sync.dma_start(scores_d[:], sc_sb[:])

    nc.compile()
    return nc


def kernel(x, adj_src, adj_dst, adj_val, edge_index, W1, b1, W2, b2):
    global LAST_RESULT
    x = np.asarray(x, np.float32)
    adj_src = np.asarray(adj_src, np.int32)
    adj_dst = np.asarray(adj_dst, np.int32)
    adj_val = np.asarray(adj_val, np.float32)
    edge_index = np.asarray(edge_index, np.int32)
    W1 = np.asarray(W1, np.float32)
    b1 = np.asarray(b1, np.float32)
    W2 = np.asarray(W2, np.float32)
    b2 = np.asarray(b2, np.float32)

    T_bc, NT, spmm_cores = _prep_spmm(adj_src, adj_dst, adj_val)
    T_b2, NT2, spmm2_cores = _prep_spmm2(adj_src, adj_dst, adj_val)
    T_wc, DEC_NT, dec_cores = _prep_decoder(edge_index)
    idx_cols = spmm_cores[0]["idx_w"].shape[1]
    idx2_cols = spmm2_cores[0]["idx2_w"].shape[1]
    dec_idx_cols = dec_cores[0]["idx_w"].shape[1]
    nc = _build(T_bc, NT, T_b2, NT2, T_wc, DEC_NT, idx_cols, idx2_cols,
                dec_idx_cols)

    xT = np.zeros((D_IN, 784 * 128), BF)
    xT[:, :N_NODES] = x.T.astype(BF)
    iota = np.tile(np.arange(128, dtype=BF)[None, :], (128, 1))
    ident = np.eye(128, dtype=BF)
    b1t = np.tile(b1.astype(BF)[None, :], (128, 1))
    b2blk = np.zeros((128, 128), BF)
    b2blk[0, 0:D_EMB] = b2.astype(BF)
    row0s = np.zeros((128, 128), BF)
    row0s[0, :] = BF(1.0)
    c32 = np.tile((np.arange(128) // 32).astype(BF)[None, :], (128, 1))
    common = {
        "W1": W1.astype(BF), "W2": W2.astype(BF),
        "b1t": b1t, "b2blk": b2blk, "row0s": row0s, "c32": c32,
        "iota": iota, "ident": ident,
    }
    in_maps = []
    for m in range(NCORE):
        in_maps.append({
            **common,
            "xT": np.ascontiguousarray(xT[:, m * ZROWS:(m + 1) * ZROWS]),
            "idx": spmm_cores[m]["idx_w"],
            "dstm": spmm_cores[m]["dst_meta"],
            "valm": spmm_cores[m]["val_meta"],
            "idx2": spmm2_cores[m]["idx2_w"],
            "dst2": spmm2_cores[m]["dst2"],
            "val2": spmm2_cores[m]["val2"],
            "par2": spmm2_cores[m]["par2"],
            "didx": dec_cores[m]["idx_w"],
            "doh": dec_cores[m]["ohdst"],
        })

    res = bass_utils.run_bass_kernel_spmd(
        nc, in_maps, core_ids=list(range(NCORE)),
        trace=bool(os.environ.get("BASS_TRACE")))
    LAST_RESULT = res

    scores = np.zeros(N_EDGES, np.float32)
    for m in range(NCORE):
        out = res.results[m]["scores"]          # [128, DEC_NT]
        emap = dec_cores[m]["emap"].reshape(DEC_NT, 128).T  # [128, DEC_NT]
        valid = emap >= 0
        scores[emap[valid]] = out[valid]
    return scores


# revision 36
# speedup vs baseline: 1.2083x; 1.2083x over previous
"""GCN link predictor on 8 TRN2 NeuronCores.

Strategy (1D node partition, dst-sharded SPMM, pull-mode gathers):
  - x is replicated (transposed, bf16); every core computes the full
    h1 = x @ W1 gather table [100096, 128] bf16 (256B rows).
  - adjacency edges are sharded by dst owner (12500 nodes/core); within a
    core, edges are grouped by (128-row dst block, 32768-row src chunk) so
    each dma_gather call uses int16 indices against a single chunk base.
  - segment-sum = PE matmul with one-hots built in BATCHED DVE
    tensor_tensor ops (broadcast APs) per superblock — avoids the ~2us
    fixed cost of per-tile tensor_scalar.
  - layer 2: h2 = z1 @ W2 locally, AllGather h2 (bf16), expand into a
    256B-row table, same SPMM -> z2 (bf16).
  - decoder: edges sharded by dst owner, sorted by (dst window, src
    chunk).  dst side = PE select from SBUF-resident local z2 windows
    (one-hot streamed from host, zero descriptors); src side = transposed
    dma_gather (A^T columns); DVE product + PE ones-reduction into PSUM
    score rows; periodic flush to SBUF.
"""
import sys
import os

sys.path.insert(0, "/opt/trn_rl_repo")

import numpy as np
import ml_dtypes
from contextlib import ExitStack

from concourse import bass, bacc, tile, bass_utils
import concourse.mybir as mybir


def _install_ntff_hook():
    """Provide antenv.axon_hooks (missing in this image) so that
    run_bass_kernel_spmd(trace=True) can capture NTFF profiles via the
    axon PJRT .so — mirrors trn_agent_boot's ctypes shim."""
    if "antenv.axon_hooks" in sys.modules:
        return
    import types, ctypes, contextlib
    import antenv

    mod = types.ModuleType("antenv.axon_hooks")
    holder = {}
    mod.set_axon_ntff_profile_hook = lambda h: holder.__setitem__("h", h)
    mod.get_axon_ntff_profile_hook = lambda: holder.get("h")
    sys.modules["antenv.axon_hooks"] = mod
    antenv.axon_hooks = mod

    so_path = "/opt/axon/libaxon_pjrt.so"
    if not os.path.exists(so_path):
        return
    lib = ctypes.CDLL(so_path)
    if not hasattr(lib, "axon_start_nrt_profile"):
        return
    lib.axon_start_nrt_profile.argtypes = [ctypes.POINTER(ctypes.c_int64),
                                           ctypes.c_size_t]
    lib.axon_start_nrt_profile.restype = ctypes.c_int64
    lib.axon_stop_nrt_profile.argtypes = [ctypes.c_char_p]
    lib.axon_stop_nrt_profile.restype = ctypes.c_int64

    @contextlib.contextmanager
    def _hook(output_dir, device_ids):
        import jax
        jax.devices()
        if device_ids:
            ids = (ctypes.c_int64 * len(device_ids))(*device_ids)
            rc = lib.axon_start_nrt_profile(ids, len(device_ids))
        else:
            rc = lib.axon_start_nrt_profile(None, 0)
        if rc != 0:
            raise RuntimeError(f"axon_start_nrt_profile rc={rc}")
        try:
            yield
        finally:
            n = lib.axon_stop_nrt_profile(str(output_dir).encode())
            print(f"profile: {n} file(s) written to {output_dir}",
                  file=sys.stderr)

    mod.set_axon_ntff_profile_hook(_hook)


_install_ntff_hook()

F32 = mybir.dt.float32
BF16 = mybir.dt.bfloat16
I16 = mybir.dt.int16
BF = ml_dtypes.bfloat16

N_NODES = 100000
D_IN = 256
D_HID = 64
D_EMB = 32
ADJ_NNZ = 3200000
N_EDGES = 2000000
NCORE = 8
P_NODES = N_NODES // NCORE          # 12500
NODES_PAD = 782 * 128               # 100096
ZROWS = 98 * 128                    # 12544 (per-core z rows, padded)
NBLK = 98                           # dst blocks per core (last has 84 rows)
CHUNK = 32768
NCHUNK = 4
SUPER = 2                           # dst blocks per superblock
NSUPER = (NBLK + SUPER - 1) // SUPER
DEC_SUPERW = 3                      # decoder dst windows per gather group
DEC_NSW = (NBLK + DEC_SUPERW - 1) // DEC_SUPERW
NPAIR4 = 25000                      # pack-4 rows of a [N,32]bf16 table

LAST_RESULT = None                  # BassKernelResults of the last run


def _wrap16(idx):
    """idx j -> partition j%16, col j//16, replicated to 128 partitions."""
    n = len(idx)
    assert n % 16 == 0
    a = idx.reshape(n // 16, 16).T
    return np.tile(a, (8, 1)).astype(np.int16)


NCH1 = 2                            # layer-1 pack-2 idx chunks (50176 rows)
PCHUNK = 32768


def _prep_spmm(adj_src, adj_dst, adj_val):
    """Layer-1 prep for the pack-2 table [50176, 128] (h1full view).

    Edges grouped by (dst block, pair-row chunk); idx = src//2 - ch*32768;
    parity stream src%2 for the masked-stationary matmul.
    """
    owner = adj_dst // P_NODES
    cores = []
    for m in range(NCORE):
        sel = owner == m
        src = adj_src[sel].astype(np.int64)
        ldst = (adj_dst[sel] - m * P_NODES).astype(np.int64)
        val = adj_val[sel]
        blk = ldst >> 7
        chk = (src // 2) // PCHUNK
        order = np.lexsort((chk, blk))
        src, ldst, val, blk, chk = (a[order] for a in (src, ldst, val, blk, chk))
        key = blk * NCH1 + chk
        cnt = np.bincount(key, minlength=NBLK * NCH1).reshape(NBLK, NCH1)
        starts = np.zeros(NBLK * NCH1 + 1, np.int64)
        np.cumsum(cnt.ravel(), out=starts[1:])
        cores.append(dict(src=src, ldst=ldst, val=val, cnt=cnt, starts=starts))

    cnt_max = np.maximum.reduce([c["cnt"] for c in cores])
    T_bc = -(-cnt_max // 128)  # ceil [98, 2]
    NT = int(T_bc.sum())

    for c in cores:
        idx_stream = np.zeros(NT * 128, np.int16)
        dst_stream = np.zeros(NT * 128, np.float32)
        val_stream = np.zeros(NT * 128, np.float32)
        par_stream = np.zeros(NT * 128, np.float32)
        pos = 0
        for sb in range(NSUPER):
            blocks = range(sb * SUPER, min((sb + 1) * SUPER, NBLK))
            for ch in range(NCH1):
                for b in blocks:
                    t = int(T_bc[b, ch])
                    if t == 0:
                        continue
                    s = c["starts"][b * NCH1 + ch]
                    e = c["starts"][b * NCH1 + ch + 1]
                    n = e - s
                    idx_stream[pos:pos + n] = (c["src"][s:e] // 2 - ch * PCHUNK).astype(np.int16)
                    dst_stream[pos:pos + n] = (c["ldst"][s:e] & 127).astype(np.float32)
                    val_stream[pos:pos + n] = c["val"][s:e]
                    par_stream[pos:pos + n] = (c["src"][s:e] % 2).astype(np.float32)
                    pos += t * 128
        assert pos == NT * 128
        # wrap idx per call (call = (sb, ch) contiguous span)
        cols = []
        p = 0
        for sb in range(NSUPER):
            blocks = range(sb * SUPER, min((sb + 1) * SUPER, NBLK))
            for ch in range(NCH1):
                t = int(T_bc[list(blocks), ch].sum())
                if t == 0:
                    continue
                cols.append(_wrap16(idx_stream[p:p + t * 128]))
                p += t * 128
        c["idx_w"] = np.concatenate(cols, axis=1)
        c["dst_meta"] = dst_stream.reshape(NT, 128).T.astype(BF).copy()
        c["val_meta"] = val_stream.reshape(NT, 128).T.astype(BF).copy()
        c["par_meta"] = par_stream.reshape(NT, 128).T.astype(BF).copy()
        for k in ("src", "ldst", "val", "cnt", "starts"):
            del c[k]
    return T_bc, NT, cores


def _prep_decoder(edge_index):
    """Shard decoder edges by dst owner; group by (dst window, src%4).

    z2 table is the pack-4 view [25000, 128] of [100000, 32] bf16; the
    gather idx is src//4 (single int16 chunk) and src%4 selects the
    32-col slice of the gathered element (static per cell).
    """
    src_g = edge_index[0].astype(np.int64)
    dst_g = edge_index[1].astype(np.int64)
    owner = dst_g // P_NODES
    cores = []
    for m in range(NCORE):
        sel = owner == m
        src = src_g[sel]
        ldst = dst_g[sel] - m * P_NODES
        eid = np.nonzero(sel)[0]
        w = ldst >> 7                      # 98 windows
        c = src % 4                        # parity cell
        sw = w // DEC_SUPERW
        order = np.lexsort((w, c, sw))
        src, ldst, eid, w, c, sw = (a[order] for a in (src, ldst, eid, w, c, sw))
        key = (w * 4 + c)
        cnt = np.bincount(key, minlength=NBLK * 4).reshape(NBLK, 4)
        cores.append(dict(src=src, ldst=ldst, eid=eid, cnt=cnt))

    cnt_max = np.maximum.reduce([c["cnt"] for c in cores])
    T_wc = -(-cnt_max // 128)              # [98, 4] tiles
    DEC_NT = int(T_wc.sum())

    for cd in cores:
        idx_stream = np.zeros(DEC_NT * 128, np.int16)
        oh = np.zeros((128, DEC_NT * 128), BF)
        emap = np.full(DEC_NT * 128, -1, np.int64)
        # per-(w,par) start offsets in this core's sorted arrays
        starts = {}
        pos0 = 0
        for isw in range(DEC_NSW):
            ws = range(isw * DEC_SUPERW, min((isw + 1) * DEC_SUPERW, NBLK))
            for par in range(4):
                for iw in ws:
                    starts[(iw, par)] = pos0
                    pos0 += int(cd["cnt"][iw, par])
        pos = 0
        for isw in range(DEC_NSW):
            ws = range(isw * DEC_SUPERW, min((isw + 1) * DEC_SUPERW, NBLK))
            for par in range(4):
                for iw in ws:
                    t = int(T_wc[iw, par])
                    if t == 0:
                        continue
                    s = starts[(iw, par)]
                    n = int(cd["cnt"][iw, par])
                    idx_stream[pos:pos + n] = (cd["src"][s:s + n] // 4).astype(np.int16)
                    loff = (cd["ldst"][s:s + n] & 127).astype(np.int64)
                    oh[loff, np.arange(pos, pos + n)] = BF(1.0)
                    emap[pos:pos + n] = cd["eid"][s:s + n]
                    pos += t * 128
        assert pos == DEC_NT * 128
        # wrap idx per (sw) call
        cols = []
        p = 0
        for isw in range(DEC_NSW):
            ws = list(range(isw * DEC_SUPERW, min((isw + 1) * DEC_SUPERW, NBLK)))
            t = int(T_wc[ws, :].sum())
            if t == 0:
                continue
            cols.append(_wrap16(idx_stream[p:p + t * 128]))
            p += t * 128
        cd["idx_w"] = np.concatenate(cols, axis=1)
        cd["ohdst"] = oh
        cd["emap"] = emap
        for k in ("src", "ldst", "eid", "cnt"):
            del cd[k]
    return T_wc, DEC_NT, cores


def _prep_spmm2(adj_src, adj_dst, adj_val):
    """Layer-2 prep for the pack-4 table: edges grouped by dst block only
    (single idx chunk, src//4 < 25000); parity stream src%4 for the
    masked-stationary matmul."""
    owner = adj_dst // P_NODES
    cores = []
    for m in range(NCORE):
        sel = owner == m
        src = adj_src[sel].astype(np.int64)
        ldst = (adj_dst[sel] - m * P_NODES).astype(np.int64)
        val = adj_val[sel]
        blk = ldst >> 7
        order = np.argsort(blk, kind="stable")
        src, ldst, val, blk = src[order], ldst[order], val[order], blk[order]
        cnt = np.bincount(blk, minlength=NBLK)
        starts = np.zeros(NBLK + 1, np.int64)
        np.cumsum(cnt, out=starts[1:])
        cores.append(dict(src=src, ldst=ldst, val=val, cnt=cnt, starts=starts))

    cnt_max = np.maximum.reduce([c["cnt"] for c in cores])
    T_b = -(-cnt_max // 128)               # [98]
    NT2 = int(T_b.sum())

    for c in cores:
        idx_stream = np.zeros(NT2 * 128, np.int16)
        dst_stream = np.zeros(NT2 * 128, np.float32)
        val_stream = np.zeros(NT2 * 128, np.float32)
        par_stream = np.zeros(NT2 * 128, np.float32)
        pos = 0
        for b in range(NBLK):
            s, e = c["starts"][b], c["starts"][b + 1]
            n = e - s
            idx_stream[pos:pos + n] = (c["src"][s:e] // 4).astype(np.int16)
            dst_stream[pos:pos + n] = (c["ldst"][s:e] & 127).astype(np.float32)
            val_stream[pos:pos + n] = c["val"][s:e]
            par_stream[pos:pos + n] = (c["src"][s:e] % 4).astype(np.float32)
            pos += int(T_b[b]) * 128
        assert pos == NT2 * 128
        cols = []
        p = 0
        for b in range(NBLK):
            t = int(T_b[b])
            if t == 0:
                continue
            cols.append(_wrap16(idx_stream[p:p + t * 128]))
            p += t * 128
        c["idx2_w"] = np.concatenate(cols, axis=1)
        c["dst2"] = dst_stream.reshape(NT2, 128).T.astype(BF).copy()
        c["val2"] = val_stream.reshape(NT2, 128).T.astype(BF).copy()
        c["par2"] = par_stream.reshape(NT2, 128).T.astype(BF).copy()
        for k in ("src", "ldst", "val", "cnt", "starts"):
            del c[k]
    return T_b, NT2, cores


def _build(T_bc, NT, T_b2, NT2, T_wc, DEC_NT, idx_cols, idx2_cols,
           dec_idx_cols):
    nc = bacc.Bacc("TRN2", target_bir_lowering=False, debug=False,
                   num_devices=NCORE)

    xT_d = nc.dram_tensor("xT", [D_IN, ZROWS], BF16, kind="ExternalInput")
    w1_d = nc.dram_tensor("W1", [D_IN, D_HID], BF16, kind="ExternalInput")
    w2_d = nc.dram_tensor("W2", [D_HID, D_EMB], BF16, kind="ExternalInput")
    b1blk_d = nc.dram_tensor("b1blk", [128, 128], BF16, kind="ExternalInput")
    b2blk_d = nc.dram_tensor("b2blk", [128, 128], BF16, kind="ExternalInput")
    row0s_d = nc.dram_tensor("row0s", [128, 128], BF16, kind="ExternalInput")
    c32_d = nc.dram_tensor("c32", [128, 128], BF16, kind="ExternalInput")
    c64_d = nc.dram_tensor("c64", [128, 128], BF16, kind="ExternalInput")
    fold32_d = nc.dram_tensor("fold32", [128, D_EMB], BF16, kind="ExternalInput")
    fold64_d = nc.dram_tensor("fold64", [128, D_HID], BF16, kind="ExternalInput")
    iota_d = nc.dram_tensor("iota", [128, 128], BF16, kind="ExternalInput")
    ident_d = nc.dram_tensor("ident", [128, 128], BF16, kind="ExternalInput")
    idx_d = nc.dram_tensor("idx", [128, idx_cols], I16, kind="ExternalInput")
    dstm_d = nc.dram_tensor("dstm", [128, NT], BF16, kind="ExternalInput")
    valm_d = nc.dram_tensor("valm", [128, NT], BF16, kind="ExternalInput")
    parm_d = nc.dram_tensor("parm", [128, NT], BF16, kind="ExternalInput")
    idx2_d = nc.dram_tensor("idx2", [128, idx2_cols], I16, kind="ExternalInput")
    dst2_d = nc.dram_tensor("dst2", [128, NT2], BF16, kind="ExternalInput")
    val2_d = nc.dram_tensor("val2", [128, NT2], BF16, kind="ExternalInput")
    par2_d = nc.dram_tensor("par2", [128, NT2], BF16, kind="ExternalInput")
    didx_d = nc.dram_tensor("didx", [128, dec_idx_cols], I16, kind="ExternalInput")
    doh_d = nc.dram_tensor("doh", [128, DEC_NT * 128], BF16, kind="ExternalInput")
    scores_d = nc.dram_tensor("scores", [128, DEC_NT], F32,
                              kind="ExternalOutput")

    # internal DRAM
    h1loc_d = nc.dram_tensor("h1loc", [ZROWS, D_HID], BF16, kind="Internal")
    h1full_d = nc.dram_tensor("h1full", [784 * 128, D_HID], BF16,
                              kind="Internal", addr_space="Shared")
    h2loc_d = nc.dram_tensor("h2loc", [ZROWS, D_EMB], BF16, kind="Internal")
    h2full_d = nc.dram_tensor("h2full", [N_NODES, D_EMB], BF16,
                              kind="Internal", addr_space="Shared")
    z2loc_d = nc.dram_tensor("z2loc", [ZROWS, D_EMB], BF16, kind="Internal")
    z2full_d = nc.dram_tensor("z2full", [N_NODES, D_EMB], BF16,
                              kind="Internal", addr_space="Shared")

    rg = [list(range(NCORE))]

    def _expand(tc, pool_, src_d, dst_d, width, dt):
        """Copy [N_NODES, width] rows into the 256B-stride table cols 0:width
        via SBUF bounce tiles."""
        R = 8192
        for r0 in range(0, N_NODES, R):
            n = min(R, N_NODES - r0)
            nb = -(-n // 128)
            t = pool_.tile([128, nb, width], dt, tag="expand")
            if n % 128:
                full = n // 128
                if full:
                    nc.sync.dma_start(
                        t[:, 0:full, :],
                        src_d[r0:r0 + full * 128, :]
                        .rearrange("(j p) f -> p j f", p=128))
                rem = n - full * 128
                nc.sync.dma_start(t[0:rem, full, :], src_d[r0 + full * 128:r0 + n, :])
                if full:
                    nc.sync.dma_start(
                        dst_d[r0:r0 + full * 128, 0:width]
                        .rearrange("(j p) f -> p j f", p=128),
                        t[:, 0:full, :])
                nc.sync.dma_start(dst_d[r0 + full * 128:r0 + n, 0:width],
                                  t[0:rem, full, :])
            else:
                nc.sync.dma_start(
                    t[:], src_d[r0:r0 + n, :]
                    .rearrange("(j p) f -> p j f", p=128))
                nc.sync.dma_start(
                    dst_d[r0:r0 + n, 0:width]
                    .rearrange("(j p) f -> p j f", p=128),
                    t[:])

    # per-call tile counts for spmm gathers
    def spmm_calls():
        out = []
        for sb in range(NSUPER):
            blocks = list(range(sb * SUPER, min((sb + 1) * SUPER, NBLK)))
            for ch in range(NCH1):
                t = int(T_bc[blocks, ch].sum())
                if t:
                    out.append((sb, ch, blocks, t))
        return out

    CALLS = spmm_calls()
    call_tile_base = {}
    tb = 0
    for (sb, ch, blocks, t) in CALLS:
        call_tile_base[(sb, ch)] = tb
        tb += t
    assert tb == NT

    def spmm1_phase(tc, pool, iota_sb, c64_sb, b1blk_sb, row0s_sb,
                    fold64_sb, table_ap, z1T_sb):
        """Layer-1 SPMM on the pack-2 table; writes relu(z1)^T into SBUF."""
        nc_ = tc.nc
        with ExitStack() as ctx:
            lp = ctx.enter_context(tc.tile_pool(name="sp_b", bufs=3))
            ohp = ctx.enter_context(tc.tile_pool(name="oh_b", bufs=2))
            gp = ctx.enter_context(tc.tile_pool(name="g_b", bufs=8))
            pp = ctx.enter_context(
                tc.tile_pool(name="ps_b", bufs=3, space="PSUM"))
            icol = 0
            for sb in range(NSUPER):
                blocks = list(range(sb * SUPER, min((sb + 1) * SUPER, NBLK)))
                sb_tiles = int(T_bc[blocks, :].sum())
                sb_tile0 = call_tile_base[(sb, [ch for ch in range(NCH1)
                                                if (sb, ch) in call_tile_base][0])]
                dst_sb = lp.tile([128, sb_tiles], BF16, tag="dstm")
                val_sb = lp.tile([128, sb_tiles], BF16, tag="valm")
                par_sb = lp.tile([128, sb_tiles], BF16, tag="parm")
                nc_.sync.dma_start(dst_sb[:], dstm_d[:, sb_tile0:sb_tile0 + sb_tiles])
                nc_.sync.dma_start(val_sb[:], valm_d[:, sb_tile0:sb_tile0 + sb_tiles])
                nc_.sync.dma_start(par_sb[:], parm_d[:, sb_tile0:sb_tile0 + sb_tiles])
                idx_sb = lp.tile([128, sb_tiles * 8], I16, tag="idx")
                nc_.sync.dma_start(idx_sb[:], idx_d[:, icol:icol + sb_tiles * 8])

                # batched one-hot + parity/val mask for the whole superblock
                oh_sb = ohp.tile([128, sb_tiles, 128], BF16, tag="oh")
                g2m = ohp.tile([128, sb_tiles, 128], BF16, tag="g2m")
                i_b = iota_sb[:].unsqueeze(1).broadcast_to([128, sb_tiles, 128])
                c_b = c64_sb[:].unsqueeze(1).broadcast_to([128, sb_tiles, 128])
                d_b = dst_sb[:].unsqueeze(2).broadcast_to([128, sb_tiles, 128])
                v_b = val_sb[:].unsqueeze(2).broadcast_to([128, sb_tiles, 128])
                p_b = par_sb[:].unsqueeze(2).broadcast_to([128, sb_tiles, 128])
                nc_.vector.tensor_tensor(oh_sb[:], i_b, d_b,
                                         mybir.AluOpType.is_equal)
                nc_.vector.tensor_tensor(g2m[:], c_b, p_b,
                                         mybir.AluOpType.is_equal)
                nc_.vector.tensor_tensor(g2m[:], g2m[:], v_b,
                                         mybir.AluOpType.mult)

                ic_local = 0
                for ch in range(NCH1):
                    if (sb, ch) not in call_tile_base:
                        continue
                    t = int(T_bc[blocks, ch].sum())
                    rows = min(PCHUNK, 50176 - ch * PCHUNK)
                    g = gp.tile([128, t, 128], BF16, tag="gath")
                    nc_.gpsimd.dma_gather(
                        out_ap=g[:],
                        in_ap=table_ap[ch * PCHUNK:ch * PCHUNK + rows, :],
                        idxs_ap=idx_sb[:, ic_local:ic_local + t * 8],
                        num_idxs=t * 128,
                        num_idxs_reg=t * 128,
                        elem_size=128,
                        single_packet=False,
                    )
                    o = call_tile_base[(sb, ch)] - sb_tile0
                    nc_.vector.tensor_tensor(
                        g2m[:, o:o + t, :], g2m[:, o:o + t, :], g[:],
                        mybir.AluOpType.mult)
                    ic_local += t * 8
                icol += sb_tiles * 8

                for bi, b in enumerate(blocks):
                    ps = pp.tile([128, 128], F32, tag="ps")
                    first = True
                    for ch in range(NCH1):
                        if (sb, ch) not in call_tile_base:
                            continue
                        off = int(T_bc[blocks[:bi], ch].sum()) if bi else 0
                        gtile0 = call_tile_base[(sb, ch)] + off
                        for ti in range(int(T_bc[b, ch])):
                            mcol = gtile0 + ti - sb_tile0
                            nc_.tensor.matmul(
                                ps[:], g2m[:, mcol, :], oh_sb[:, mcol, :],
                                start=first, stop=False)
                            first = False
                    nc_.tensor.matmul(ps[:], b1blk_sb[:], row0s_sb[:],
                                      start=first, stop=True)
                    pcop = lp.tile([128, 128], BF16, tag="pcop")
                    nc_.scalar.activation(pcop[:], ps[:],
                                          mybir.ActivationFunctionType.Copy)
                    zps = pp.tile([D_HID, 128], F32, tag="zps")
                    nc_.tensor.matmul(zps[:], fold64_sb[:], pcop[:],
                                      start=True, stop=True)
                    nc_.scalar.activation(z1T_sb[:, b * 128:(b + 1) * 128],
                                          zps[:],
                                          mybir.ActivationFunctionType.Relu)

    with tile.TileContext(nc) as tc:
        with ExitStack() as octx:
            pool = octx.enter_context(tc.tile_pool(name="const", bufs=1))
            iota_sb = pool.tile([128, 128], BF16, tag="iota")
            ident_sb = pool.tile([128, 128], BF16, tag="ident")
            nc.sync.dma_start(iota_sb[:], iota_d[:])
            nc.sync.dma_start(ident_sb[:], ident_d[:])

            # ---- Phase A: h1 shard = x[:, my 98 blocks] @ W1; AllGather ----
            with ExitStack() as ctx:
                ap = ctx.enter_context(tc.tile_pool(name="pA", bufs=3))
                app = ctx.enter_context(
                    tc.tile_pool(name="pAp", bufs=8, space="PSUM"))
                w1_sb = pool.tile([128, 2, D_HID], BF16, tag="w1")
                nc.sync.dma_start(
                    w1_sb[:], w1_d[:].rearrange("(k p) f -> p k f", p=128))
                PB = 8  # node blocks per panel
                for p0 in range(0, NBLK, PB):
                    nb = min(PB, NBLK - p0)
                    n0 = p0 * 128
                    xt0 = ap.tile([128, nb * 128], BF16, tag="xt0")
                    xt1 = ap.tile([128, nb * 128], BF16, tag="xt1")
                    nc.sync.dma_start(xt0[:], xT_d[0:128, n0:n0 + nb * 128])
                    nc.sync.dma_start(xt1[:], xT_d[128:256, n0:n0 + nb * 128])
                    hw = ap.tile([128, nb, D_HID], BF16, tag="hw")
                    for j in range(nb):
                        ps = app.tile([128, D_HID], F32, tag="psA")
                        nc.tensor.matmul(ps[:], xt0[:, j * 128:(j + 1) * 128],
                                         w1_sb[:, 0, :], start=True, stop=False)
                        nc.tensor.matmul(ps[:], xt1[:, j * 128:(j + 1) * 128],
                                         w1_sb[:, 1, :], start=False, stop=True)
                        nc.scalar.activation(hw[:, j, :], ps[:],
                                             mybir.ActivationFunctionType.Copy)
                    nc.sync.dma_start(
                        h1loc_d[p0 * 128:(p0 + nb) * 128, :]
                        .rearrange("(j p) f -> p j f", p=128),
                        hw[:],
                    )
                nc.gpsimd.collective_compute(
                    "AllGather", mybir.AluOpType.bypass, replica_groups=rg,
                    ins=[h1loc_d[:]], outs=[h1full_d.ap()])

            # ---- Phase B: SPMM1 -> z1T (relu) via pack-2 h1full view ----
            c64_sb = pool.tile([128, 128], BF16, tag="c64")
            b1blk_sb = pool.tile([128, 128], BF16, tag="b1blk")
            row0s_sb = pool.tile([128, 128], BF16, tag="row0s")
            nc.sync.dma_start(c64_sb[:], c64_d[:])
            nc.sync.dma_start(b1blk_sb[:], b1blk_d[:])
            nc.sync.dma_start(row0s_sb[:], row0s_d[:])
            z1T_sb = pool.tile([D_HID, ZROWS], BF16, tag="z1T")
            tab1_ap = h1full_d[:].rearrange("(j k) f -> j (k f)", k=2)
            fold64_sb = pool.tile([128, D_HID], BF16, tag="fold64")
            nc.sync.dma_start(fold64_sb[:], fold64_d[:])
            spmm1_phase(tc, pool, iota_sb, c64_sb, b1blk_sb, row0s_sb,
                        fold64_sb, tab1_ap, z1T_sb)

            # ---- Phase C: h2 = z1 @ W2; AllGather; expand to table2 ----
            with ExitStack() as ctx:
                cp = ctx.enter_context(tc.tile_pool(name="pC", bufs=3))
                cpp = ctx.enter_context(
                    tc.tile_pool(name="pCp", bufs=4, space="PSUM"))
                w2_sb = pool.tile([D_HID, D_EMB], BF16, tag="w2")
                nc.sync.dma_start(w2_sb[:], w2_d[:])
                PB = 8
                for p0 in range(0, NBLK, PB):
                    nb = min(PB, NBLK - p0)
                    hw = cp.tile([128, nb, D_EMB], BF16, tag="h2w")
                    for j in range(nb):
                        b = p0 + j
                        ps = cpp.tile([128, D_EMB], F32, tag="psC")
                        nc.tensor.matmul(ps[:],
                                         z1T_sb[:, b * 128:(b + 1) * 128],
                                         w2_sb[:], start=True, stop=True)
                        nc.scalar.activation(hw[:, j, :], ps[:],
                                             mybir.ActivationFunctionType.Copy)
                    nc.sync.dma_start(
                        h2loc_d[p0 * 128:(p0 + nb) * 128, :]
                        .rearrange("(j p) f -> p j f", p=128),
                        hw[:],
                    )
                nc.gpsimd.collective_compute(
                    "AllGather", mybir.AluOpType.bypass, replica_groups=rg,
                    ins=[h2loc_d[0:P_NODES, :]], outs=[h2full_d.ap()])

            # ---- Phase D: SPMM2 -> z2 via pack-4 table (h2full view) ----
            tab2_ap = h2full_d[:].rearrange("(j k) f -> j (k f)", k=4)
            with ExitStack() as ctx:
                lp = ctx.enter_context(tc.tile_pool(name="sp_d", bufs=3))
                ohp = ctx.enter_context(tc.tile_pool(name="oh_d", bufs=2))
                gp = ctx.enter_context(tc.tile_pool(name="g_d", bufs=6))
                pp = ctx.enter_context(
                    tc.tile_pool(name="ps_d", bufs=3, space="PSUM"))
                c32_sb = pool.tile([128, 128], BF16, tag="c32")
                b2blk_sb = pool.tile([128, 128], BF16, tag="b2blk")
                fold32_sb = pool.tile([128, D_EMB], BF16, tag="fold32")
                nc.sync.dma_start(c32_sb[:], c32_d[:])
                nc.sync.dma_start(b2blk_sb[:], b2blk_d[:])
                nc.sync.dma_start(fold32_sb[:], fold32_d[:])
                icol2 = 0
                tpos2 = 0
                PB = 4
                zwide = None
                for b in range(NBLK):
                    t = int(T_b2[b])
                    dst_sb = lp.tile([128, t], BF16, tag="dst2")
                    val_sb = lp.tile([128, t], BF16, tag="val2")
                    par_sb = lp.tile([128, t], BF16, tag="par2")
                    nc.sync.dma_start(dst_sb[:], dst2_d[:, tpos2:tpos2 + t])
                    nc.sync.dma_start(val_sb[:], val2_d[:, tpos2:tpos2 + t])
                    nc.sync.dma_start(par_sb[:], par2_d[:, tpos2:tpos2 + t])
                    idx_sb = lp.tile([128, t * 8], I16, tag="idx2")
                    nc.sync.dma_start(idx_sb[:], idx2_d[:, icol2:icol2 + t * 8])
                    icol2 += t * 8
                    g4 = gp.tile([128, t, 128], BF16, tag="g4")
                    nc.gpsimd.dma_gather(
                        out_ap=g4[:], in_ap=tab2_ap[0:NPAIR4, :],
                        idxs_ap=idx_sb[:], num_idxs=t * 128,
                        num_idxs_reg=t * 128, elem_size=128,
                        single_packet=False)
                    # batched DVE: binary dst one-hot + parity/val mask
                    ohb = ohp.tile([128, t, 128], BF16, tag="ohb")
                    g4m = ohp.tile([128, t, 128], BF16, tag="g4m")
                    i_b = iota_sb[:].unsqueeze(1).broadcast_to([128, t, 128])
                    c_b = c32_sb[:].unsqueeze(1).broadcast_to([128, t, 128])
                    d_b = dst_sb[:].unsqueeze(2).broadcast_to([128, t, 128])
                    v_b = val_sb[:].unsqueeze(2).broadcast_to([128, t, 128])
                    p_b = par_sb[:].unsqueeze(2).broadcast_to([128, t, 128])
                    nc.vector.tensor_tensor(ohb[:], i_b, d_b,
                                            mybir.AluOpType.is_equal)
                    nc.vector.tensor_tensor(g4m[:], c_b, p_b,
                                            mybir.AluOpType.is_equal)
                    nc.vector.tensor_tensor(g4m[:], g4m[:], v_b,
                                            mybir.AluOpType.mult)
                    nc.vector.tensor_tensor(g4m[:], g4m[:], g4[:],
                                            mybir.AluOpType.mult)
                    ps = pp.tile([128, 128], F32, tag="psD")
                    for ti in range(t):
                        nc.tensor.matmul(ps[:], g4m[:, ti, :], ohb[:, ti, :],
                                         start=(ti == 0), stop=False)
                    nc.tensor.matmul(ps[:], b2blk_sb[:], row0s_sb[:],
                                     start=False, stop=True)
                    # z2T_blk[f, d] = sum_q ps[q*32+f, d]; transpose to [d, f]
                    pcop = lp.tile([128, 128], BF16, tag="pcop")
                    nc.scalar.activation(pcop[:], ps[:],
                                         mybir.ActivationFunctionType.Copy)
                    # z2_blk[d, f] = sum_c pcop[c, d] * fold32[c, f]
                    ztps = pp.tile([128, D_EMB], F32, tag="ztps")
                    nc.tensor.matmul(ztps[:], pcop[:], fold32_sb[:],
                                     start=True, stop=True)
                    if b % PB == 0:
                        zwide = lp.tile([128, PB, D_EMB], BF16, tag="zw2")
                    nc.scalar.activation(zwide[:, b % PB, :], ztps[:],
                                         mybir.ActivationFunctionType.Copy)
                    if b % PB == PB - 1 or b == NBLK - 1:
                        b0 = (b // PB) * PB
                        nbw = b - b0 + 1
                        nc.sync.dma_start(
                            z2loc_d[b0 * 128:(b0 + nbw) * 128, :]
                            .rearrange("(j p) f -> p j f", p=128),
                            zwide[:, 0:nbw, :],
                        )
                    tpos2 += t

            # AllGather z2 (bf16); decoder reads the pack-4 view directly
            nc.gpsimd.collective_compute(
                "AllGather", mybir.AluOpType.bypass, replica_groups=rg,
                ins=[z2loc_d[0:P_NODES, :]], outs=[z2full_d.ap()])
            tab3_ap = z2full_d[:].rearrange("(j k) f -> j (k f)", k=4)

            # ---- Phase E: decoder ----
            GRP = 8
            with ExitStack() as ctx:
                ep = ctx.enter_context(tc.tile_pool(name="pE", bufs=3))
                egp = ctx.enter_context(tc.tile_pool(name="pEg", bufs=4))
                epp = ctx.enter_context(
                    tc.tile_pool(name="pEp", bufs=4, space="PSUM"))
                # local z2 windows, partition = ldst & 127
                z2w_sb = pool.tile([128, NBLK, D_EMB], BF16, tag="z2w")
                nc.sync.dma_start(
                    z2w_sb[:],
                    z2loc_d[:].rearrange("(j p) f -> p j f", p=128))
                # scores accumulate in SBUF [128, DEC_NT] (edge = t*128 + p)
                sc_sb = pool.tile([128, DEC_NT], F32, tag="scsb")

                tile_pos = 0          # global tile counter (stream order)
                icol = 0
                for isw in range(DEC_NSW):
                    ws = list(range(isw * DEC_SUPERW,
                                    min((isw + 1) * DEC_SUPERW, NBLK)))
                    t = int(T_wc[ws, :].sum())
                    if t == 0:
                        continue
                    didx_sb = ep.tile([128, t * 8], I16, tag="didx")
                    nc.sync.dma_start(didx_sb[:],
                                      didx_d[:, icol:icol + t * 8])
                    icol += t * 8
                    at = egp.tile([128, t, 128], BF16, tag="at")
                    nc.gpsimd.dma_gather(
                        out_ap=at[:],
                        in_ap=tab3_ap[0:NPAIR4, :],
                        idxs_ap=didx_sb[:],
                        num_idxs=t * 128, num_idxs_reg=t * 128,
                        elem_size=128, single_packet=False)
                    oh_sb = ep.tile([128, t * 128], BF16, tag="doh")
                    nc.sync.dma_start(
                        oh_sb[:],
                        doh_d[:, tile_pos * 128:(tile_pos + t) * 128])
                    # (window, parity) of each tile in this call, stream order
                    tl = 0
                    for par in range(4):
                        for iw in ws:
                            tw = int(T_wc[iw, par])
                            if tw == 0:
                                continue
                            for g0 in range(0, tw, GRP):
                                gn = min(GRP, tw - g0)
                                bt2 = epp.tile([128, GRP, D_EMB], F32,
                                               tag="bt2")
                                for j in range(gn):
                                    tj = tl + g0 + j
                                    nc.tensor.matmul(
                                        bt2[:, j, :],
                                        oh_sb[:, tj * 128:(tj + 1) * 128],
                                        z2w_sb[:, iw, :],
                                        start=True, stop=True)
                                prod = ep.tile([128, GRP, D_EMB], BF16,
                                               tag="prod")
                                nc.vector.tensor_tensor(
                                    prod[:, 0:gn, :],
                                    at[:, tl + g0:tl + g0 + gn,
                                       par * D_EMB:(par + 1) * D_EMB],
                                    bt2[:, 0:gn, :], mybir.AluOpType.mult)
                                nc.vector.tensor_reduce(
                                    sc_sb[:, tile_pos + tl + g0:
                                          tile_pos + tl + g0 + gn],
                                    prod[:, 0:gn, :], mybir.AxisListType.X,
                                    mybir.AluOpType.add)
                            tl += tw
                    assert tl == t
                    tile_pos += t
                nc.sync.dma_start(scores_d[:], sc_sb[:])

    nc.compile()
    return nc


def kernel(x, adj_src, adj_dst, adj_val, edge_index, W1, b1, W2, b2):
    global LAST_RESULT
    x = np.asarray(x, np.float32)
    adj_src = np.asarray(adj_src, np.int32)
    adj_dst = np.asarray(adj_dst, np.int32)
    adj_val = np.asarray(adj_val, np.float32)
    edge_index = np.asarray(edge_index, np.int32)
    W1 = np.asarray(W1, np.float32)
    b1 = np.asarray(b1, np.float32)
    W2 = np.asarray(W2, np.float32)
    b2 = np.asarray(b2, np.float32)

    T_bc, NT, spmm_cores = _prep_spmm(adj_src, adj_dst, adj_val)
    T_b2, NT2, spmm2_cores = _prep_spmm2(adj_src, adj_dst, adj_val)
    T_wc, DEC_NT, dec_cores = _prep_decoder(edge_index)
    idx_cols = spmm_cores[0]["idx_w"].shape[1]
    idx2_cols = spmm2_cores[0]["idx2_w"].shape[1]
    dec_idx_cols = dec_cores[0]["idx_w"].shape[1]
    nc = _build(T_bc, NT, T_b2, NT2, T_wc, DEC_NT, idx_cols, idx2_cols,
                dec_idx_cols)

    xT = np.zeros((D_IN, 784 * 128), BF)
    xT[:, :N_NODES] = x.T.astype(BF)
    iota = np.tile(np.arange(128, dtype=BF)[None, :], (128, 1))
    ident = np.eye(128, dtype=BF)
    b1blk = np.zeros((128, 128), BF)
    b1blk[0, 0:D_HID] = b1.astype(BF)
    b2blk = np.zeros((128, 128), BF)
    b2blk[0, 0:D_EMB] = b2.astype(BF)
    row0s = np.zeros((128, 128), BF)
    row0s[0, :] = BF(1.0)
    c32 = np.tile((np.arange(128) // 32).astype(BF)[None, :], (128, 1))
    c64 = np.tile((np.arange(128) // 64).astype(BF)[None, :], (128, 1))
    fold32 = (np.arange(128)[:, None] % 32 == np.arange(D_EMB)[None, :]).astype(BF)
    fold64 = (np.arange(128)[:, None] % 64 == np.arange(D_HID)[None, :]).astype(BF)
    common = {
        "W1": W1.astype(BF), "W2": W2.astype(BF),
        "b1blk": b1blk, "b2blk": b2blk, "row0s": row0s, "c32": c32,
        "c64": c64, "fold32": fold32, "fold64": fold64,
        "iota": iota, "ident": ident,
    }
    in_maps = []
    for m in range(NCORE):
        in_maps.append({
            **common,
            "xT": np.ascontiguousarray(xT[:, m * ZROWS:(m + 1) * ZROWS]),
            "idx": spmm_cores[m]["idx_w"],
            "dstm": spmm_cores[m]["dst_meta"],
            "valm": spmm_cores[m]["val_meta"],
            "parm": spmm_cores[m]["par_meta"],
            "idx2": spmm2_cores[m]["idx2_w"],
            "dst2": spmm2_cores[m]["dst2"],
            "val2": spmm2_cores[m]["val2"],
            "par2": spmm2_cores[m]["par2"],
            "didx": dec_cores[m]["idx_w"],
            "doh": dec_cores[m]["ohdst"],
        })

    res = bass_utils.run_bass_kernel_spmd(
        nc, in_maps, core_ids=list(range(NCORE)),
        trace=bool(os.environ.get("BASS_TRACE")))
    LAST_RESULT = res

    scores = np.zeros(N_EDGES, np.float32)
    for m in range(NCORE):
        out = res.results[m]["scores"]          # [128, DEC_NT]
        emap = dec_cores[m]["emap"].reshape(DEC_NT, 128).T  # [128, DEC_NT]
        valid = emap >= 0
        scores[emap[valid]] = out[valid]
    return scores


# revision 37
# speedup vs baseline: 1.2147x; 1.0053x over previous
"""GCN link predictor on 8 TRN2 NeuronCores.

The kernel is bound by SWDGE dma_gather descriptor generation on the Pool
engine (~8ns per gathered element, independent of element size/locality),
so the design minimizes gathered-element count and keeps every other
engine hidden underneath:

  - Phase A: x is shard-split by node block (12544/core); each core
    computes its h1 = x @ W1 slice, AllGather -> h1full [100352, 64] bf16.
    The layer-1 gather table is the PACK-2 VIEW [50176, 128] of those
    bytes (256B elements = 2 nodes) — no expand/copy.
  - SPMM1: edges sharded by dst owner, grouped by (dst block, pair
    chunk).  Per edge one 256B gather (idx = src//2, int16).  Segment-sum
    via PE: stationary = gathered tile masked by parity(src)*val (batched
    DVE tensor_tensor with broadcast APs), moving = binary dst one-hot;
    PSUM accumulates [q*64+f, d] per block; a constant fold64 matmul sums
    the parity halves -> z1^T kept in SBUF (relu via ACT).
  - Phase C: h2 = z1 @ W2 from SBUF-resident z1^T; AllGather h2full
    [100000, 32] bf16.  Layer-2 table = PACK-4 VIEW [25000, 128] (single
    idx chunk, 98 dst-block cells only).
  - SPMM2: same masked-stationary scheme with 4-way parity (c32 mask);
    fold via pcop^T @ fold32 matmul -> z2 node-major, AllGather z2full.
  - decoder: edges sharded by dst owner, cells (dst window, src%4).
    dst side = PE select from SBUF-resident local z2 windows (binary
    one-hot streamed from host, zero descriptors); src side = one 256B
    gather per edge from the pack-4 z2full view; DVE product (static
    src%4 slice) + tensor_reduce -> scores [128, DEC_NT].
"""
import sys
import os

sys.path.insert(0, "/opt/trn_rl_repo")

import numpy as np
import ml_dtypes
from contextlib import ExitStack

from concourse import bass, bacc, tile, bass_utils
import concourse.mybir as mybir


def _install_ntff_hook():
    """Provide antenv.axon_hooks (missing in this image) so that
    run_bass_kernel_spmd(trace=True) can capture NTFF profiles via the
    axon PJRT .so — mirrors trn_agent_boot's ctypes shim."""
    if "antenv.axon_hooks" in sys.modules:
        return
    import types, ctypes, contextlib
    import antenv

    mod = types.ModuleType("antenv.axon_hooks")
    holder = {}
    mod.set_axon_ntff_profile_hook = lambda h: holder.__setitem__("h", h)
    mod.get_axon_ntff_profile_hook = lambda: holder.get("h")
    sys.modules["antenv.axon_hooks"] = mod
    antenv.axon_hooks = mod

    so_path = "/opt/axon/libaxon_pjrt.so"
    if not os.path.exists(so_path):
        return
    lib = ctypes.CDLL(so_path)
    if not hasattr(lib, "axon_start_nrt_profile"):
        return
    lib.axon_start_nrt_profile.argtypes = [ctypes.POINTER(ctypes.c_int64),
                                           ctypes.c_size_t]
    lib.axon_start_nrt_profile.restype = ctypes.c_int64
    lib.axon_stop_nrt_profile.argtypes = [ctypes.c_char_p]
    lib.axon_stop_nrt_profile.restype = ctypes.c_int64

    @contextlib.contextmanager
    def _hook(output_dir, device_ids):
        import jax
        jax.devices()
        if device_ids:
            ids = (ctypes.c_int64 * len(device_ids))(*device_ids)
            rc = lib.axon_start_nrt_profile(ids, len(device_ids))
        else:
            rc = lib.axon_start_nrt_profile(None, 0)
        if rc != 0:
            raise RuntimeError(f"axon_start_nrt_profile rc={rc}")
        try:
            yield
        finally:
            n = lib.axon_stop_nrt_profile(str(output_dir).encode())
            print(f"profile: {n} file(s) written to {output_dir}",
                  file=sys.stderr)

    mod.set_axon_ntff_profile_hook(_hook)


_install_ntff_hook()

F32 = mybir.dt.float32
BF16 = mybir.dt.bfloat16
I16 = mybir.dt.int16
BF = ml_dtypes.bfloat16

N_NODES = 100000
D_IN = 256
D_HID = 64
D_EMB = 32
ADJ_NNZ = 3200000
N_EDGES = 2000000
NCORE = 8
P_NODES = N_NODES // NCORE          # 12500
NODES_PAD = 782 * 128               # 100096
ZROWS = 98 * 128                    # 12544 (per-core z rows, padded)
NBLK = 98                           # dst blocks per core (last has 84 rows)
CHUNK = 32768
NCHUNK = 4
SUPER = 2                           # dst blocks per superblock
NSUPER = (NBLK + SUPER - 1) // SUPER
DEC_SUPERW = 3                      # decoder dst windows per gather group
DEC_NSW = (NBLK + DEC_SUPERW - 1) // DEC_SUPERW
NPAIR4 = 25000                      # pack-4 rows of a [N,32]bf16 table

LAST_RESULT = None                  # BassKernelResults of the last run


def _wrap16(idx):
    """idx j -> partition j%16, col j//16, replicated to 128 partitions."""
    n = len(idx)
    assert n % 16 == 0
    a = idx.reshape(n // 16, 16).T
    return np.tile(a, (8, 1)).astype(np.int16)


NCH1 = 2                            # layer-1 pack-2 idx chunks (50176 rows)
PCHUNK = 32768


def _prep_spmm(adj_src, adj_dst, adj_val):
    """Layer-1 prep for the pack-2 table [50176, 128] (h1full view).

    Edges grouped by (dst block, pair-row chunk); idx = src//2 - ch*32768;
    parity stream src%2 for the masked-stationary matmul.
    """
    owner = adj_dst // P_NODES
    cores = []
    for m in range(NCORE):
        sel = owner == m
        src = adj_src[sel].astype(np.int64)
        ldst = (adj_dst[sel] - m * P_NODES).astype(np.int64)
        val = adj_val[sel]
        blk = ldst >> 7
        chk = (src // 2) // PCHUNK
        order = np.lexsort((chk, blk))
        src, ldst, val, blk, chk = (a[order] for a in (src, ldst, val, blk, chk))
        key = blk * NCH1 + chk
        cnt = np.bincount(key, minlength=NBLK * NCH1).reshape(NBLK, NCH1)
        starts = np.zeros(NBLK * NCH1 + 1, np.int64)
        np.cumsum(cnt.ravel(), out=starts[1:])
        cores.append(dict(src=src, ldst=ldst, val=val, cnt=cnt, starts=starts))

    cnt_max = np.maximum.reduce([c["cnt"] for c in cores])
    T_bc = -(-cnt_max // 128)  # ceil [98, 2]
    NT = int(T_bc.sum())

    for c in cores:
        idx_stream = np.zeros(NT * 128, np.int16)
        dst_stream = np.zeros(NT * 128, np.float32)
        val_stream = np.zeros(NT * 128, np.float32)
        par_stream = np.zeros(NT * 128, np.float32)
        pos = 0
        for sb in range(NSUPER):
            blocks = range(sb * SUPER, min((sb + 1) * SUPER, NBLK))
            for ch in range(NCH1):
                for b in blocks:
                    t = int(T_bc[b, ch])
                    if t == 0:
                        continue
                    s = c["starts"][b * NCH1 + ch]
                    e = c["starts"][b * NCH1 + ch + 1]
                    n = e - s
                    idx_stream[pos:pos + n] = (c["src"][s:e] // 2 - ch * PCHUNK).astype(np.int16)
                    dst_stream[pos:pos + n] = (c["ldst"][s:e] & 127).astype(np.float32)
                    val_stream[pos:pos + n] = c["val"][s:e]
                    par_stream[pos:pos + n] = (c["src"][s:e] % 2).astype(np.float32)
                    pos += t * 128
        assert pos == NT * 128
        # wrap idx per call (call = (sb, ch) contiguous span)
        cols = []
        p = 0
        for sb in range(NSUPER):
            blocks = range(sb * SUPER, min((sb + 1) * SUPER, NBLK))
            for ch in range(NCH1):
                t = int(T_bc[list(blocks), ch].sum())
                if t == 0:
                    continue
                cols.append(_wrap16(idx_stream[p:p + t * 128]))
                p += t * 128
        c["idx_w"] = np.concatenate(cols, axis=1)
        c["dst_meta"] = dst_stream.reshape(NT, 128).T.astype(BF).copy()
        c["val_meta"] = val_stream.reshape(NT, 128).T.astype(BF).copy()
        c["par_meta"] = par_stream.reshape(NT, 128).T.astype(BF).copy()
        for k in ("src", "ldst", "val", "cnt", "starts"):
            del c[k]
    return T_bc, NT, cores


def _prep_decoder(edge_index):
    """Shard decoder edges by dst owner; group by (dst window, src%4).

    z2 table is the pack-4 view [25000, 128] of [100000, 32] bf16; the
    gather idx is src//4 (single int16 chunk) and src%4 selects the
    32-col slice of the gathered element (static per cell).
    """
    src_g = edge_index[0].astype(np.int64)
    dst_g = edge_index[1].astype(np.int64)
    owner = dst_g // P_NODES
    cores = []
    for m in range(NCORE):
        sel = owner == m
        src = src_g[sel]
        ldst = dst_g[sel] - m * P_NODES
        eid = np.nonzero(sel)[0]
        w = ldst >> 7                      # 98 windows
        c = src % 4                        # parity cell
        sw = w // DEC_SUPERW
        order = np.lexsort((w, c, sw))
        src, ldst, eid, w, c, sw = (a[order] for a in (src, ldst, eid, w, c, sw))
        key = (w * 4 + c)
        cnt = np.bincount(key, minlength=NBLK * 4).reshape(NBLK, 4)
        cores.append(dict(src=src, ldst=ldst, eid=eid, cnt=cnt))

    cnt_max = np.maximum.reduce([c["cnt"] for c in cores])
    T_wc = -(-cnt_max // 128)              # [98, 4] tiles
    DEC_NT = int(T_wc.sum())

    for cd in cores:
        idx_stream = np.zeros(DEC_NT * 128, np.int16)
        oh = np.zeros((128, DEC_NT * 128), BF)
        emap = np.full(DEC_NT * 128, -1, np.int64)
        # per-(w,par) start offsets in this core's sorted arrays
        starts = {}
        pos0 = 0
        for isw in range(DEC_NSW):
            ws = range(isw * DEC_SUPERW, min((isw + 1) * DEC_SUPERW, NBLK))
            for par in range(4):
                for iw in ws:
                    starts[(iw, par)] = pos0
                    pos0 += int(cd["cnt"][iw, par])
        pos = 0
        for isw in range(DEC_NSW):
            ws = range(isw * DEC_SUPERW, min((isw + 1) * DEC_SUPERW, NBLK))
            for par in range(4):
                for iw in ws:
                    t = int(T_wc[iw, par])
                    if t == 0:
                        continue
                    s = starts[(iw, par)]
                    n = int(cd["cnt"][iw, par])
                    idx_stream[pos:pos + n] = (cd["src"][s:s + n] // 4).astype(np.int16)
                    loff = (cd["ldst"][s:s + n] & 127).astype(np.int64)
                    oh[loff, np.arange(pos, pos + n)] = BF(1.0)
                    emap[pos:pos + n] = cd["eid"][s:s + n]
                    pos += t * 128
        assert pos == DEC_NT * 128
        # wrap idx per (sw) call
        cols = []
        p = 0
        for isw in range(DEC_NSW):
            ws = list(range(isw * DEC_SUPERW, min((isw + 1) * DEC_SUPERW, NBLK)))
            t = int(T_wc[ws, :].sum())
            if t == 0:
                continue
            cols.append(_wrap16(idx_stream[p:p + t * 128]))
            p += t * 128
        cd["idx_w"] = np.concatenate(cols, axis=1)
        cd["ohdst"] = oh
        cd["emap"] = emap
        for k in ("src", "ldst", "eid", "cnt"):
            del cd[k]
    return T_wc, DEC_NT, cores


def _prep_spmm2(adj_src, adj_dst, adj_val):
    """Layer-2 prep for the pack-4 table: edges grouped by dst block only
    (single idx chunk, src//4 < 25000); parity stream src%4 for the
    masked-stationary matmul."""
    owner = adj_dst // P_NODES
    cores = []
    for m in range(NCORE):
        sel = owner == m
        src = adj_src[sel].astype(np.int64)
        ldst = (adj_dst[sel] - m * P_NODES).astype(np.int64)
        val = adj_val[sel]
        blk = ldst >> 7
        order = np.argsort(blk, kind="stable")
        src, ldst, val, blk = src[order], ldst[order], val[order], blk[order]
        cnt = np.bincount(blk, minlength=NBLK)
        starts = np.zeros(NBLK + 1, np.int64)
        np.cumsum(cnt, out=starts[1:])
        cores.append(dict(src=src, ldst=ldst, val=val, cnt=cnt, starts=starts))

    cnt_max = np.maximum.reduce([c["cnt"] for c in cores])
    T_b = -(-cnt_max // 128)               # [98]
    NT2 = int(T_b.sum())

    for c in cores:
        idx_stream = np.zeros(NT2 * 128, np.int16)
        dst_stream = np.zeros(NT2 * 128, np.float32)
        val_stream = np.zeros(NT2 * 128, np.float32)
        par_stream = np.zeros(NT2 * 128, np.float32)
        pos = 0
        for b in range(NBLK):
            s, e = c["starts"][b], c["starts"][b + 1]
            n = e - s
            idx_stream[pos:pos + n] = (c["src"][s:e] // 4).astype(np.int16)
            dst_stream[pos:pos + n] = (c["ldst"][s:e] & 127).astype(np.float32)
            val_stream[pos:pos + n] = c["val"][s:e]
            par_stream[pos:pos + n] = (c["src"][s:e] % 4).astype(np.float32)
            pos += int(T_b[b]) * 128
        assert pos == NT2 * 128
        cols = []
        p = 0
        for b in range(NBLK):
            t = int(T_b[b])
            if t == 0:
                continue
            cols.append(_wrap16(idx_stream[p:p + t * 128]))
            p += t * 128
        c["idx2_w"] = np.concatenate(cols, axis=1)
        c["dst2"] = dst_stream.reshape(NT2, 128).T.astype(BF).copy()
        c["val2"] = val_stream.reshape(NT2, 128).T.astype(BF).copy()
        c["par2"] = par_stream.reshape(NT2, 128).T.astype(BF).copy()
        for k in ("src", "ldst", "val", "cnt", "starts"):
            del c[k]
    return T_b, NT2, cores


def _build(T_bc, NT, T_b2, NT2, T_wc, DEC_NT, idx_cols, idx2_cols,
           dec_idx_cols):
    nc = bacc.Bacc("TRN2", target_bir_lowering=False, debug=False,
                   num_devices=NCORE)

    xT_d = nc.dram_tensor("xT", [D_IN, ZROWS], BF16, kind="ExternalInput")
    w1_d = nc.dram_tensor("W1", [D_IN, D_HID], BF16, kind="ExternalInput")
    w2_d = nc.dram_tensor("W2", [D_HID, D_EMB], BF16, kind="ExternalInput")
    b1blk_d = nc.dram_tensor("b1blk", [128, 128], BF16, kind="ExternalInput")
    b2blk_d = nc.dram_tensor("b2blk", [128, 128], BF16, kind="ExternalInput")
    row0s_d = nc.dram_tensor("row0s", [128, 128], BF16, kind="ExternalInput")
    c32_d = nc.dram_tensor("c32", [128, 128], BF16, kind="ExternalInput")
    c64_d = nc.dram_tensor("c64", [128, 128], BF16, kind="ExternalInput")
    fold32_d = nc.dram_tensor("fold32", [128, D_EMB], BF16, kind="ExternalInput")
    fold64_d = nc.dram_tensor("fold64", [128, D_HID], BF16, kind="ExternalInput")
    iota_d = nc.dram_tensor("iota", [128, 128], BF16, kind="ExternalInput")
    ident_d = nc.dram_tensor("ident", [128, 128], BF16, kind="ExternalInput")
    idx_d = nc.dram_tensor("idx", [128, idx_cols], I16, kind="ExternalInput")
    dstm_d = nc.dram_tensor("dstm", [128, NT], BF16, kind="ExternalInput")
    valm_d = nc.dram_tensor("valm", [128, NT], BF16, kind="ExternalInput")
    parm_d = nc.dram_tensor("parm", [128, NT], BF16, kind="ExternalInput")
    idx2_d = nc.dram_tensor("idx2", [128, idx2_cols], I16, kind="ExternalInput")
    dst2_d = nc.dram_tensor("dst2", [128, NT2], BF16, kind="ExternalInput")
    val2_d = nc.dram_tensor("val2", [128, NT2], BF16, kind="ExternalInput")
    par2_d = nc.dram_tensor("par2", [128, NT2], BF16, kind="ExternalInput")
    didx_d = nc.dram_tensor("didx", [128, dec_idx_cols], I16, kind="ExternalInput")
    doh_d = nc.dram_tensor("doh", [128, DEC_NT * 128], BF16, kind="ExternalInput")
    scores_d = nc.dram_tensor("scores", [128, DEC_NT], F32,
                              kind="ExternalOutput")

    # internal DRAM
    h1loc_d = nc.dram_tensor("h1loc", [ZROWS, D_HID], BF16, kind="Internal")
    h1full_d = nc.dram_tensor("h1full", [784 * 128, D_HID], BF16,
                              kind="Internal", addr_space="Shared")
    h2loc_d = nc.dram_tensor("h2loc", [ZROWS, D_EMB], BF16, kind="Internal")
    h2full_d = nc.dram_tensor("h2full", [N_NODES, D_EMB], BF16,
                              kind="Internal", addr_space="Shared")
    z2loc_d = nc.dram_tensor("z2loc", [ZROWS, D_EMB], BF16, kind="Internal")
    z2full_d = nc.dram_tensor("z2full", [N_NODES, D_EMB], BF16,
                              kind="Internal", addr_space="Shared")

    rg = [list(range(NCORE))]

    def _expand(tc, pool_, src_d, dst_d, width, dt):
        """Copy [N_NODES, width] rows into the 256B-stride table cols 0:width
        via SBUF bounce tiles."""
        R = 8192
        for r0 in range(0, N_NODES, R):
            n = min(R, N_NODES - r0)
            nb = -(-n // 128)
            t = pool_.tile([128, nb, width], dt, tag="expand")
            if n % 128:
                full = n // 128
                if full:
                    nc.sync.dma_start(
                        t[:, 0:full, :],
                        src_d[r0:r0 + full * 128, :]
                        .rearrange("(j p) f -> p j f", p=128))
                rem = n - full * 128
                nc.sync.dma_start(t[0:rem, full, :], src_d[r0 + full * 128:r0 + n, :])
                if full:
                    nc.sync.dma_start(
                        dst_d[r0:r0 + full * 128, 0:width]
                        .rearrange("(j p) f -> p j f", p=128),
                        t[:, 0:full, :])
                nc.sync.dma_start(dst_d[r0 + full * 128:r0 + n, 0:width],
                                  t[0:rem, full, :])
            else:
                nc.sync.dma_start(
                    t[:], src_d[r0:r0 + n, :]
                    .rearrange("(j p) f -> p j f", p=128))
                nc.sync.dma_start(
                    dst_d[r0:r0 + n, 0:width]
                    .rearrange("(j p) f -> p j f", p=128),
                    t[:])

    # per-call tile counts for spmm gathers
    def spmm_calls():
        out = []
        for sb in range(NSUPER):
            blocks = list(range(sb * SUPER, min((sb + 1) * SUPER, NBLK)))
            for ch in range(NCH1):
                t = int(T_bc[blocks, ch].sum())
                if t:
                    out.append((sb, ch, blocks, t))
        return out

    CALLS = spmm_calls()
    call_tile_base = {}
    tb = 0
    for (sb, ch, blocks, t) in CALLS:
        call_tile_base[(sb, ch)] = tb
        tb += t
    assert tb == NT

    def spmm1_phase(tc, pool, iota_sb, c64_sb, b1blk_sb, row0s_sb,
                    fold64_sb, table_ap, z1T_sb):
        """Layer-1 SPMM on the pack-2 table; writes relu(z1)^T into SBUF."""
        nc_ = tc.nc
        with ExitStack() as ctx:
            lp = ctx.enter_context(tc.tile_pool(name="sp_b", bufs=3))
            ohp = ctx.enter_context(tc.tile_pool(name="oh_b", bufs=2))
            gp = ctx.enter_context(tc.tile_pool(name="g_b", bufs=8))
            pp = ctx.enter_context(
                tc.tile_pool(name="ps_b", bufs=3, space="PSUM"))
            icol = 0
            for sb in range(NSUPER):
                blocks = list(range(sb * SUPER, min((sb + 1) * SUPER, NBLK)))
                sb_tiles = int(T_bc[blocks, :].sum())
                sb_tile0 = call_tile_base[(sb, [ch for ch in range(NCH1)
                                                if (sb, ch) in call_tile_base][0])]
                dst_sb = lp.tile([128, sb_tiles], BF16, tag="dstm")
                val_sb = lp.tile([128, sb_tiles], BF16, tag="valm")
                par_sb = lp.tile([128, sb_tiles], BF16, tag="parm")
                nc_.sync.dma_start(dst_sb[:], dstm_d[:, sb_tile0:sb_tile0 + sb_tiles])
                nc_.sync.dma_start(val_sb[:], valm_d[:, sb_tile0:sb_tile0 + sb_tiles])
                nc_.sync.dma_start(par_sb[:], parm_d[:, sb_tile0:sb_tile0 + sb_tiles])
                idx_sb = lp.tile([128, sb_tiles * 8], I16, tag="idx")
                nc_.sync.dma_start(idx_sb[:], idx_d[:, icol:icol + sb_tiles * 8])

                # batched one-hot + parity/val mask for the whole superblock
                oh_sb = ohp.tile([128, sb_tiles, 128], BF16, tag="oh")
                g2m = ohp.tile([128, sb_tiles, 128], BF16, tag="g2m")
                i_b = iota_sb[:].unsqueeze(1).broadcast_to([128, sb_tiles, 128])
                c_b = c64_sb[:].unsqueeze(1).broadcast_to([128, sb_tiles, 128])
                d_b = dst_sb[:].unsqueeze(2).broadcast_to([128, sb_tiles, 128])
                v_b = val_sb[:].unsqueeze(2).broadcast_to([128, sb_tiles, 128])
                p_b = par_sb[:].unsqueeze(2).broadcast_to([128, sb_tiles, 128])
                nc_.vector.tensor_tensor(oh_sb[:], i_b, d_b,
                                         mybir.AluOpType.is_equal)
                nc_.vector.tensor_tensor(g2m[:], c_b, p_b,
                                         mybir.AluOpType.is_equal)
                nc_.vector.tensor_tensor(g2m[:], g2m[:], v_b,
                                         mybir.AluOpType.mult)

                ic_local = 0
                for ch in range(NCH1):
                    if (sb, ch) not in call_tile_base:
                        continue
                    t = int(T_bc[blocks, ch].sum())
                    rows = min(PCHUNK, 50176 - ch * PCHUNK)
                    g = gp.tile([128, t, 128], BF16, tag="gath")
                    nc_.gpsimd.dma_gather(
                        out_ap=g[:],
                        in_ap=table_ap[ch * PCHUNK:ch * PCHUNK + rows, :],
                        idxs_ap=idx_sb[:, ic_local:ic_local + t * 8],
                        num_idxs=t * 128,
                        num_idxs_reg=t * 128,
                        elem_size=128,
                        single_packet=False,
                    )
                    o = call_tile_base[(sb, ch)] - sb_tile0
                    nc_.vector.tensor_tensor(
                        g2m[:, o:o + t, :], g2m[:, o:o + t, :], g[:],
                        mybir.AluOpType.mult)
                    ic_local += t * 8
                icol += sb_tiles * 8

                for bi, b in enumerate(blocks):
                    ps = pp.tile([128, 128], F32, tag="ps")
                    first = True
                    for ch in range(NCH1):
                        if (sb, ch) not in call_tile_base:
                            continue
                        off = int(T_bc[blocks[:bi], ch].sum()) if bi else 0
                        gtile0 = call_tile_base[(sb, ch)] + off
                        for ti in range(int(T_bc[b, ch])):
                            mcol = gtile0 + ti - sb_tile0
                            nc_.tensor.matmul(
                                ps[:], g2m[:, mcol, :], oh_sb[:, mcol, :],
                                start=first, stop=False)
                            first = False
                    nc_.tensor.matmul(ps[:], b1blk_sb[:], row0s_sb[:],
                                      start=first, stop=True)
                    pcop = lp.tile([128, 128], BF16, tag="pcop")
                    nc_.scalar.activation(pcop[:], ps[:],
                                          mybir.ActivationFunctionType.Copy)
                    zps = pp.tile([D_HID, 128], F32, tag="zps")
                    nc_.tensor.matmul(zps[:], fold64_sb[:], pcop[:],
                                      start=True, stop=True)
                    nc_.scalar.activation(z1T_sb[:, b * 128:(b + 1) * 128],
                                          zps[:],
                                          mybir.ActivationFunctionType.Relu)

    with tile.TileContext(nc) as tc:
        with ExitStack() as octx:
            pool = octx.enter_context(tc.tile_pool(name="const", bufs=1))
            iota_sb = pool.tile([128, 128], BF16, tag="iota")
            ident_sb = pool.tile([128, 128], BF16, tag="ident")
            nc.sync.dma_start(iota_sb[:], iota_d[:])
            nc.sync.dma_start(ident_sb[:], ident_d[:])

            # ---- Phase A: h1 shard = x[:, my 98 blocks] @ W1; AllGather ----
            with ExitStack() as ctx:
                ap = ctx.enter_context(tc.tile_pool(name="pA", bufs=3))
                app = ctx.enter_context(
                    tc.tile_pool(name="pAp", bufs=8, space="PSUM"))
                w1_sb = pool.tile([128, 2, D_HID], BF16, tag="w1")
                nc.sync.dma_start(
                    w1_sb[:], w1_d[:].rearrange("(k p) f -> p k f", p=128))
                PB = 8  # node blocks per panel
                for p0 in range(0, NBLK, PB):
                    nb = min(PB, NBLK - p0)
                    n0 = p0 * 128
                    xt0 = ap.tile([128, nb * 128], BF16, tag="xt0")
                    xt1 = ap.tile([128, nb * 128], BF16, tag="xt1")
                    nc.sync.dma_start(xt0[:], xT_d[0:128, n0:n0 + nb * 128])
                    nc.sync.dma_start(xt1[:], xT_d[128:256, n0:n0 + nb * 128])
                    hw = ap.tile([128, nb, D_HID], BF16, tag="hw")
                    for j in range(nb):
                        ps = app.tile([128, D_HID], F32, tag="psA")
                        nc.tensor.matmul(ps[:], xt0[:, j * 128:(j + 1) * 128],
                                         w1_sb[:, 0, :], start=True, stop=False)
                        nc.tensor.matmul(ps[:], xt1[:, j * 128:(j + 1) * 128],
                                         w1_sb[:, 1, :], start=False, stop=True)
                        nc.scalar.activation(hw[:, j, :], ps[:],
                                             mybir.ActivationFunctionType.Copy)
                    nc.sync.dma_start(
                        h1loc_d[p0 * 128:(p0 + nb) * 128, :]
                        .rearrange("(j p) f -> p j f", p=128),
                        hw[:],
                    )
                nc.gpsimd.collective_compute(
                    "AllGather", mybir.AluOpType.bypass, replica_groups=rg,
                    ins=[h1loc_d[:]], outs=[h1full_d.ap()])

            # ---- Phase B: SPMM1 -> z1T (relu) via pack-2 h1full view ----
            c64_sb = pool.tile([128, 128], BF16, tag="c64")
            b1blk_sb = pool.tile([128, 128], BF16, tag="b1blk")
            row0s_sb = pool.tile([128, 128], BF16, tag="row0s")
            nc.sync.dma_start(c64_sb[:], c64_d[:])
            nc.sync.dma_start(b1blk_sb[:], b1blk_d[:])
            nc.sync.dma_start(row0s_sb[:], row0s_d[:])
            z1T_sb = pool.tile([D_HID, ZROWS], BF16, tag="z1T")
            tab1_ap = h1full_d[:].rearrange("(j k) f -> j (k f)", k=2)
            fold64_sb = pool.tile([128, D_HID], BF16, tag="fold64")
            nc.sync.dma_start(fold64_sb[:], fold64_d[:])
            spmm1_phase(tc, pool, iota_sb, c64_sb, b1blk_sb, row0s_sb,
                        fold64_sb, tab1_ap, z1T_sb)

            # ---- Phase C: h2 = z1 @ W2; AllGather; expand to table2 ----
            with ExitStack() as ctx:
                cp = ctx.enter_context(tc.tile_pool(name="pC", bufs=3))
                cpp = ctx.enter_context(
                    tc.tile_pool(name="pCp", bufs=4, space="PSUM"))
                w2_sb = pool.tile([D_HID, D_EMB], BF16, tag="w2")
                nc.sync.dma_start(w2_sb[:], w2_d[:])
                PB = 8
                for p0 in range(0, NBLK, PB):
                    nb = min(PB, NBLK - p0)
                    hw = cp.tile([128, nb, D_EMB], BF16, tag="h2w")
                    for j in range(nb):
                        b = p0 + j
                        ps = cpp.tile([128, D_EMB], F32, tag="psC")
                        nc.tensor.matmul(ps[:],
                                         z1T_sb[:, b * 128:(b + 1) * 128],
                                         w2_sb[:], start=True, stop=True)
                        nc.scalar.activation(hw[:, j, :], ps[:],
                                             mybir.ActivationFunctionType.Copy)
                    nc.sync.dma_start(
                        h2loc_d[p0 * 128:(p0 + nb) * 128, :]
                        .rearrange("(j p) f -> p j f", p=128),
                        hw[:],
                    )
                nc.gpsimd.collective_compute(
                    "AllGather", mybir.AluOpType.bypass, replica_groups=rg,
                    ins=[h2loc_d[0:P_NODES, :]], outs=[h2full_d.ap()])

            # ---- Phase D: SPMM2 -> z2 via pack-4 table (h2full view) ----
            tab2_ap = h2full_d[:].rearrange("(j k) f -> j (k f)", k=4)
            with ExitStack() as ctx:
                lp = ctx.enter_context(tc.tile_pool(name="sp_d", bufs=3))
                ohp = ctx.enter_context(tc.tile_pool(name="oh_d", bufs=2))
                gp = ctx.enter_context(tc.tile_pool(name="g_d", bufs=6))
                pp = ctx.enter_context(
                    tc.tile_pool(name="ps_d", bufs=3, space="PSUM"))
                c32_sb = pool.tile([128, 128], BF16, tag="c32")
                b2blk_sb = pool.tile([128, 128], BF16, tag="b2blk")
                fold32_sb = pool.tile([128, D_EMB], BF16, tag="fold32")
                nc.sync.dma_start(c32_sb[:], c32_d[:])
                nc.sync.dma_start(b2blk_sb[:], b2blk_d[:])
                nc.sync.dma_start(fold32_sb[:], fold32_d[:])
                icol2 = 0
                tpos2 = 0
                PB = 4
                zwide = None
                for b in range(NBLK):
                    t = int(T_b2[b])
                    dst_sb = lp.tile([128, t], BF16, tag="dst2")
                    val_sb = lp.tile([128, t], BF16, tag="val2")
                    par_sb = lp.tile([128, t], BF16, tag="par2")
                    nc.sync.dma_start(dst_sb[:], dst2_d[:, tpos2:tpos2 + t])
                    nc.sync.dma_start(val_sb[:], val2_d[:, tpos2:tpos2 + t])
                    nc.sync.dma_start(par_sb[:], par2_d[:, tpos2:tpos2 + t])
                    idx_sb = lp.tile([128, t * 8], I16, tag="idx2")
                    nc.sync.dma_start(idx_sb[:], idx2_d[:, icol2:icol2 + t * 8])
                    icol2 += t * 8
                    g4 = gp.tile([128, t, 128], BF16, tag="g4")
                    nc.gpsimd.dma_gather(
                        out_ap=g4[:], in_ap=tab2_ap[0:NPAIR4, :],
                        idxs_ap=idx_sb[:], num_idxs=t * 128,
                        num_idxs_reg=t * 128, elem_size=128,
                        single_packet=False)
                    # batched DVE: binary dst one-hot + parity/val mask
                    ohb = ohp.tile([128, t, 128], BF16, tag="ohb")
                    g4m = ohp.tile([128, t, 128], BF16, tag="g4m")
                    i_b = iota_sb[:].unsqueeze(1).broadcast_to([128, t, 128])
                    c_b = c32_sb[:].unsqueeze(1).broadcast_to([128, t, 128])
                    d_b = dst_sb[:].unsqueeze(2).broadcast_to([128, t, 128])
                    v_b = val_sb[:].unsqueeze(2).broadcast_to([128, t, 128])
                    p_b = par_sb[:].unsqueeze(2).broadcast_to([128, t, 128])
                    nc.vector.tensor_tensor(ohb[:], i_b, d_b,
                                            mybir.AluOpType.is_equal)
                    nc.vector.tensor_tensor(g4m[:], c_b, p_b,
                                            mybir.AluOpType.is_equal)
                    nc.vector.tensor_tensor(g4m[:], g4m[:], v_b,
                                            mybir.AluOpType.mult)
                    nc.vector.tensor_tensor(g4m[:], g4m[:], g4[:],
                                            mybir.AluOpType.mult)
                    ps = pp.tile([128, 128], F32, tag="psD")
                    for ti in range(t):
                        nc.tensor.matmul(ps[:], g4m[:, ti, :], ohb[:, ti, :],
                                         start=(ti == 0), stop=False)
                    nc.tensor.matmul(ps[:], b2blk_sb[:], row0s_sb[:],
                                     start=False, stop=True)
                    # z2T_blk[f, d] = sum_q ps[q*32+f, d]; transpose to [d, f]
                    pcop = lp.tile([128, 128], BF16, tag="pcop")
                    nc.scalar.activation(pcop[:], ps[:],
                                         mybir.ActivationFunctionType.Copy)
                    # z2_blk[d, f] = sum_c pcop[c, d] * fold32[c, f]
                    ztps = pp.tile([128, D_EMB], F32, tag="ztps")
                    nc.tensor.matmul(ztps[:], pcop[:], fold32_sb[:],
                                     start=True, stop=True)
                    if b % PB == 0:
                        zwide = lp.tile([128, PB, D_EMB], BF16, tag="zw2")
                    nc.scalar.activation(zwide[:, b % PB, :], ztps[:],
                                         mybir.ActivationFunctionType.Copy)
                    if b % PB == PB - 1 or b == NBLK - 1:
                        b0 = (b // PB) * PB
                        nbw = b - b0 + 1
                        nc.sync.dma_start(
                            z2loc_d[b0 * 128:(b0 + nbw) * 128, :]
                            .rearrange("(j p) f -> p j f", p=128),
                            zwide[:, 0:nbw, :],
                        )
                    tpos2 += t

            # AllGather z2 (bf16); decoder reads the pack-4 view directly
            nc.gpsimd.collective_compute(
                "AllGather", mybir.AluOpType.bypass, replica_groups=rg,
                ins=[z2loc_d[0:P_NODES, :]], outs=[z2full_d.ap()])
            tab3_ap = z2full_d[:].rearrange("(j k) f -> j (k f)", k=4)

            # ---- Phase E: decoder ----
            GRP = 8
            with ExitStack() as ctx:
                ep = ctx.enter_context(tc.tile_pool(name="pE", bufs=3))
                egp = ctx.enter_context(tc.tile_pool(name="pEg", bufs=4))
                epp = ctx.enter_context(
                    tc.tile_pool(name="pEp", bufs=4, space="PSUM"))
                # local z2 windows, partition = ldst & 127
                z2w_sb = pool.tile([128, NBLK, D_EMB], BF16, tag="z2w")
                nc.sync.dma_start(
                    z2w_sb[:],
                    z2loc_d[:].rearrange("(j p) f -> p j f", p=128))
                # scores accumulate in SBUF [128, DEC_NT] (edge = t*128 + p)
                sc_sb = pool.tile([128, DEC_NT], F32, tag="scsb")

                tile_pos = 0          # global tile counter (stream order)
                icol = 0
                for isw in range(DEC_NSW):
                    ws = list(range(isw * DEC_SUPERW,
                                    min((isw + 1) * DEC_SUPERW, NBLK)))
                    t = int(T_wc[ws, :].sum())
                    if t == 0:
                        continue
                    didx_sb = ep.tile([128, t * 8], I16, tag="didx")
                    nc.sync.dma_start(didx_sb[:],
                                      didx_d[:, icol:icol + t * 8])
                    icol += t * 8
                    at = egp.tile([128, t, 128], BF16, tag="at")
                    nc.gpsimd.dma_gather(
                        out_ap=at[:],
                        in_ap=tab3_ap[0:NPAIR4, :],
                        idxs_ap=didx_sb[:],
                        num_idxs=t * 128, num_idxs_reg=t * 128,
                        elem_size=128, single_packet=False)
                    oh_sb = ep.tile([128, t * 128], BF16, tag="doh")
                    nc.sync.dma_start(
                        oh_sb[:],
                        doh_d[:, tile_pos * 128:(tile_pos + t) * 128])
                    # (window, parity) of each tile in this call, stream order
                    tl = 0
                    for par in range(4):
                        for iw in ws:
                            tw = int(T_wc[iw, par])
                            if tw == 0:
                                continue
                            for g0 in range(0, tw, GRP):
                                gn = min(GRP, tw - g0)
                                bt2 = epp.tile([128, GRP, D_EMB], F32,
                                               tag="bt2")
                                for j in range(gn):
                                    tj = tl + g0 + j
                                    nc.tensor.matmul(
                                        bt2[:, j, :],
                                        oh_sb[:, tj * 128:(tj + 1) * 128],
                                        z2w_sb[:, iw, :],
                                        start=True, stop=True)
                                prod = ep.tile([128, GRP, D_EMB], BF16,
                                               tag="prod")
                                nc.vector.tensor_tensor(
                                    prod[:, 0:gn, :],
                                    at[:, tl + g0:tl + g0 + gn,
                                       par * D_EMB:(par + 1) * D_EMB],
                                    bt2[:, 0:gn, :], mybir.AluOpType.mult)
                                nc.vector.tensor_reduce(
                                    sc_sb[:, tile_pos + tl + g0:
                                          tile_pos + tl + g0 + gn],
                                    prod[:, 0:gn, :], mybir.AxisListType.X,
                                    mybir.AluOpType.add)
                            tl += tw
                    assert tl == t
                    tile_pos += t
                nc.sync.dma_start(scores_d[:], sc_sb[:])

    nc.compile()
    return nc


def kernel(x, adj_src, adj_dst, adj_val, edge_index, W1, b1, W2, b2):
    global LAST_RESULT
    x = np.asarray(x, np.float32)
    adj_src = np.asarray(adj_src, np.int32)
    adj_dst = np.asarray(adj_dst, np.int32)
    adj_val = np.asarray(adj_val, np.float32)
    edge_index = np.asarray(edge_index, np.int32)
    W1 = np.asarray(W1, np.float32)
    b1 = np.asarray(b1, np.float32)
    W2 = np.asarray(W2, np.float32)
    b2 = np.asarray(b2, np.float32)

    T_bc, NT, spmm_cores = _prep_spmm(adj_src, adj_dst, adj_val)
    T_b2, NT2, spmm2_cores = _prep_spmm2(adj_src, adj_dst, adj_val)
    T_wc, DEC_NT, dec_cores = _prep_decoder(edge_index)
    idx_cols = spmm_cores[0]["idx_w"].shape[1]
    idx2_cols = spmm2_cores[0]["idx2_w"].shape[1]
    dec_idx_cols = dec_cores[0]["idx_w"].shape[1]
    nc = _build(T_bc, NT, T_b2, NT2, T_wc, DEC_NT, idx_cols, idx2_cols,
                dec_idx_cols)

    xT = np.zeros((D_IN, 784 * 128), BF)
    xT[:, :N_NODES] = x.T.astype(BF)
    iota = np.tile(np.arange(128, dtype=BF)[None, :], (128, 1))
    ident = np.eye(128, dtype=BF)
    b1blk = np.zeros((128, 128), BF)
    b1blk[0, 0:D_HID] = b1.astype(BF)
    b2blk = np.zeros((128, 128), BF)
    b2blk[0, 0:D_EMB] = b2.astype(BF)
    row0s = np.zeros((128, 128), BF)
    row0s[0, :] = BF(1.0)
    c32 = np.tile((np.arange(128) // 32).astype(BF)[None, :], (128, 1))
    c64 = np.tile((np.arange(128) // 64).astype(BF)[None, :], (128, 1))
    fold32 = (np.arange(128)[:, None] % 32 == np.arange(D_EMB)[None, :]).astype(BF)
    fold64 = (np.arange(128)[:, None] % 64 == np.arange(D_HID)[None, :]).astype(BF)
    common = {
        "W1": W1.astype(BF), "W2": W2.astype(BF),
        "b1blk": b1blk, "b2blk": b2blk, "row0s": row0s, "c32": c32,
        "c64": c64, "fold32": fold32, "fold64": fold64,
        "iota": iota, "ident": ident,
    }
    in_maps = []
    for m in range(NCORE):
        in_maps.append({
            **common,
            "xT": np.ascontiguousarray(xT[:, m * ZROWS:(m + 1) * ZROWS]),
            "idx": spmm_cores[m]["idx_w"],
            "dstm": spmm_cores[m]["dst_meta"],
            "valm": spmm_cores[m]["val_meta"],
            "parm": spmm_cores[m]["par_meta"],
            "idx2": spmm2_cores[m]["idx2_w"],
            "dst2": spmm2_cores[m]["dst2"],
            "val2": spmm2_cores[m]["val2"],
            "par2": spmm2_cores[m]["par2"],
            "didx": dec_cores[m]["idx_w"],
            "doh": dec_cores[m]["ohdst"],
        })

    res = bass_utils.run_bass_kernel_spmd(
        nc, in_maps, core_ids=list(range(NCORE)),
        trace=bool(os.environ.get("BASS_TRACE")))
    LAST_RESULT = res

    scores = np.zeros(N_EDGES, np.float32)
    for m in range(NCORE):
        out = res.results[m]["scores"]          # [128, DEC_NT]
        emap = dec_cores[m]["emap"].reshape(DEC_NT, 128).T  # [128, DEC_NT]
        valid = emap >= 0
        scores[emap[valid]] = out[valid]
    return scores
